# revision 1
# baseline (speedup 1.0000x reference)
"""
CrossAttention kernel for 8x Trainium2 NeuronCores (Bass/Tile).

Problem: nn_CrossAttention (B=4, Sq=Skv=1024, DM=1024, H=16, DK=64), fp32 I/O.

Sharding (Megatron-style hybrid, 8 cores = 2 batch-pairs x 4 head-quarters):
  core c -> batches {2*(c//4), 2*(c//4)+1}, heads {4*(c%4) .. 4*(c%4)+3}.
Each core computes Q/K/V projections for its 4 heads (256 of the 1024
Wq/Wk/Wv output dims), attention for those heads, and a partial O
projection (its 256 rows of Wo).  The host sums the 4 head-quarter
partials per batch and adds bo + bv@Wo (bias-after-allreduce; bv is
dropped on device since softmax weights sum to 1).

Device-side dataflow (per core), all matmuls bf16 with fp32 PSUM accumulate:
  - Host pre-transposes y1/y2 (feature-major yT, cast bf16) and ships
    ebT = exp(attn_bias)^T bf16 [HC, Skv, Sq]; no on-device casts needed.
  - qT/kT produced feature-major [dk, Sq]; bq (pre-scaled by 1/sqrt(dk)) and
    bk fused into the PSUM->SBUF evacuation via ACT activation bias.
  - v produced token-major [Skv, dk]; a ones-column is appended so the PV
    matmul emits the softmax denominator as PSUM row 64 for free (M=65).
  - logits computed transposed l^T [Skv, Sq] = k^T q; exp on ACT straight to
    bf16 in SBUF (no max-subtraction: |logits| <= ~15 is exact-exp-safe);
    the bias enters multiplicatively: p~ = exp(l) * exp(bias) via a DVE
    scalar_tensor_tensor multiply (4x mode: all-SBUF packed bf16).
  - attn^T = v^T p~^T (unnormalized) + denominator row; normalization applied
    at PSUM evacuation: r = 1/s broadcast across partitions via a DRAM
    round-trip DMA (0-step partition AP), DVE multiply during evacuation.
  - O projection token-major from assembled attn^T head-pair tiles.
"""

import os
import sys

sys.path.insert(0, "/opt/trn_rl_repo")

from contextlib import ExitStack

import numpy as np
import ml_dtypes

import concourse.bass as bass
import concourse.mybir as mybir
import concourse.tile as tile
from concourse import bacc
from concourse.bass_utils import run_bass_kernel_spmd

BF16 = mybir.dt.bfloat16
F32 = mybir.dt.float32
AF = mybir.ActivationFunctionType
ALU = mybir.AluOpType

B, Sq, Skv, DM = 4, 1024, 1024, 1024
H, DK = 16, 64
HC = 4            # heads per core
DH = HC * DK      # head dims per core (256)
BC = 2            # batches per core
SCALE = DK ** -0.5
N_CORES = 8

_PROGRAM = None   # cached (nc, out_name)
TRACE = bool(int(os.environ.get("BASS_KERNEL_TRACE", "0")))
LPS_BUFS = int(os.environ.get("K_LPS_BUFS", "2"))
APS_BUFS = int(os.environ.get("K_APS_BUFS", "4"))
OPS_BUFS = int(os.environ.get("K_OPS_BUFS", "2"))
MULT_POOL = int(os.environ.get("K_MULT_POOL", "3"))  # kt whose exp-mult goes to gpsimd
LAST_RESULTS = None


def build_program():
    """Build the per-core SPMD Bass program (identical on all 8 cores)."""
    nc = bacc.Bacc(
        "TRN2",
        target_bir_lowering=False,
        debug=False,
        num_devices=N_CORES,
    )

    # ---- DRAM parameters (per-core shards, host-prepared) ----
    y1T = nc.dram_tensor("y1T", [BC, DM, Skv], BF16, kind="ExternalInput")
    y2T = nc.dram_tensor("y2T", [BC, DM, Sq], BF16, kind="ExternalInput")
    ebT = nc.dram_tensor("ebT", [HC, Skv, Sq], BF16, kind="ExternalInput")  # biasT bf16
    idn = nc.dram_tensor("idn", [128, 128], BF16, kind="ExternalInput")  # identity
    wq = nc.dram_tensor("wq", [DM, DH], BF16, kind="ExternalInput")
    wk = nc.dram_tensor("wk", [DM, DH], BF16, kind="ExternalInput")
    wv = nc.dram_tensor("wv", [DM, DH], BF16, kind="ExternalInput")
    wo = nc.dram_tensor("wo", [DH, DM], BF16, kind="ExternalInput")
    bqv = nc.dram_tensor("bqv", [128, 2], F32, kind="ExternalInput")   # bq*SCALE, col-chunked
    bkv = nc.dram_tensor("bkv", [128, 2], F32, kind="ExternalInput")   # bk, col-chunked
    out = nc.dram_tensor("out", [BC, Sq, DM], BF16, kind="ExternalOutput")

    with tile.TileContext(nc) as tc, ExitStack() as ctx:
        build_kernel(ctx, tc, y1T, y2T, ebT, wq, wk, wv, wo, bqv, bkv, idn, out)

    nc.compile()
    return nc, "out"


def build_kernel(ctx, tc, y1T, y2T, ebT, wq, wk, wv, wo, bqv, bkv, idn, out):
    nc = tc.nc
    KT = DM // 128            # 8 contraction tiles for projections
    KG = 2                    # k-tiles per y DMA group
    NG = KT // KG             # 4 groups
    NQ = Sq // 512            # 2 moving-dim halves
    MS = Skv // 128           # 8 skv row tiles

    # ---------------- constant / weight loads ----------------
    consts = ctx.enter_context(tc.tile_pool(name="consts", bufs=1))

    wq_sb = consts.tile([128, KT, DH], BF16, tag="wq", name="wq")
    wk_sb = consts.tile([128, KT, DH], BF16, tag="wk", name="wk")
    wv_sb = consts.tile([128, KT, DH], BF16, tag="wv", name="wv")
    wo_sb = consts.tile([128, 2, DM], BF16, tag="wo", name="wo")   # [256,1024] -> 2 k-tiles
    bq_sb = consts.tile([128, 2], F32, tag="bq", name="bq")
    bk_sb = consts.tile([128, 2], F32, tag="bk", name="bk")
    idn_sb = consts.tile([128, 128], BF16, tag="idn", name="idn")
    ones_sb = consts.tile([128, 1], BF16, tag="ones", name="ones")
    nc.vector.memset(ones_sb[:], 1.0)

    # wq/bq issued first so the Q projection can start ASAP (wq in two chunks
    # for finer arrival granularity); the rest of the weight loads are
    # interleaved into the phase-P DMA stream (HWDGE is a serialized
    # resource, so issue order = arrival order).
    wq_r = wq.ap().rearrange("(k p) m -> p k m", p=128)
    nc.sync.dma_start(out=wq_sb[:, 0:2, :], in_=wq_r[:, 0:2, :])
    nc.sync.dma_start(out=bq_sb[:], in_=bqv[:, :])
    nc.sync.dma_start(out=wq_sb[:, 2:KT, :], in_=wq_r[:, 2:KT, :])

    def load_late_weights():
        nc.sync.dma_start(out=wv_sb[:], in_=wv.ap().rearrange("(k p) m -> p k m", p=128))
        nc.sync.dma_start(out=wo_sb[:], in_=wo.ap().rearrange("(k p) m -> p k m", p=128))
        nc.sync.dma_start(out=idn_sb[:], in_=idn[:, :])

    def load_k_weights():
        nc.sync.dma_start(out=wk_sb[:], in_=wk.ap().rearrange("(k p) m -> p k m", p=128))
        nc.sync.dma_start(out=bk_sb[:], in_=bkv[:, :])

    # persistent activations (both batches)
    acts = ctx.enter_context(tc.tile_pool(name="acts", bufs=1))
    qT_sb = [[acts.tile([128, Sq], BF16, tag=f"qT{b}{p}", name=f"qT{b}{p}") for p in range(2)]
             for b in range(BC)]                       # [b][head-pair] rows=2x64 dk
    kT_sb = [[acts.tile([128, Skv], BF16, tag=f"kT{b}{p}", name=f"kT{b}{p}") for p in range(2)]
             for b in range(BC)]
    v_sb = [[acts.tile([128, HC, DK], BF16, tag=f"v{b}{m}", name=f"v{b}{m}") for m in range(MS)]
            for b in range(BC)]                        # [b][skv-tile][head][dk]
    pair_sb = [[acts.tile([128, Sq], BF16, tag=f"at{b}{p}", name=f"at{b}{p}") for p in range(2)]
               for b in range(BC)]                     # attnT head-pair tiles

    # ---------------- Phase P: batch-0 Q/K projections ----------------
    # Batch-1's Q/K (and both V projections) are deferred into batch-0's
    # attention steps, where the ACT-paced exp stream leaves the PE idle.
    y1g_all = {}
    y2g_all = {}
    ybf1 = ctx.enter_context(tc.tile_pool(name="ybf1", bufs=BC * NG))
    ybf = ctx.enter_context(tc.tile_pool(name="ybf", bufs=BC * NG))
    def load_y(dram, pool, b, tag):
        gs = []
        for g in range(NG):
            t = pool.tile([128, KG, Sq], BF16, tag=tag, name=tag)
            nc.sync.dma_start(
                out=t[:],
                in_=dram[b, 128 * KG * g:128 * KG * (g + 1), :]
                .rearrange("(k p) q -> p k q", p=128),
            )
            gs.append(t)
        return gs

    with ExitStack() as pctx:
        ppsum = pctx.enter_context(tc.tile_pool(name="ppsum", bufs=4, space="PSUM"))

        y2g_all[0] = load_y(y2T, ybf, 0, "ybf")
        load_k_weights()
        qps = {(m, n): ppsum.tile([128, 512], F32, tag="ps", name="ps")
               for m in range(2) for n in range(NQ)}
        for k in range(KT):           # k-outer: start as soon as group lands
            for m in range(2):
                for n in range(NQ):
                    nc.tensor.matmul(
                        qps[(m, n)][:],
                        lhsT=wq_sb[:, k, 128 * m:128 * (m + 1)],
                        rhs=y2g_all[0][k // KG][:, k % KG, 512 * n:512 * (n + 1)],
                        start=(k == 0), stop=(k == KT - 1),
                    )
        for m in range(2):
            for n in range(NQ):
                nc.scalar.activation(
                    qT_sb[0][m][:, 512 * n:512 * (n + 1)], qps[(m, n)][:],
                    AF.Identity, bias=bq_sb[:, m:m + 1], scale=SCALE,
                )
        y1g_all[0] = load_y(y1T, ybf1, 0, "ybf1")
        load_late_weights()
        kps = {(m, n): ppsum.tile([128, 512], F32, tag="ps", name="ps")
               for m in range(2) for n in range(NQ)}
        for k in range(KT):
            for m in range(2):
                for n in range(NQ):
                    nc.tensor.matmul(
                        kps[(m, n)][:],
                        lhsT=wk_sb[:, k, 128 * m:128 * (m + 1)],
                        rhs=y1g_all[0][k // KG][:, k % KG, 512 * n:512 * (n + 1)],
                        start=(k == 0), stop=(k == KT - 1),
                    )
        for m in range(2):
            for n in range(NQ):
                nc.scalar.activation(
                    kT_sb[0][m][:, 512 * n:512 * (n + 1)], kps[(m, n)][:],
                    AF.Identity, bias=bk_sb[:, m:m + 1], scale=1.0,
                )
        # batch-1 y loads are issued inside the attention phase (steps 0-1)
        # to keep them out of the startup DMA flood

    # ---------------- Phase A: attention + interleaved O projection ----------------
    with ExitStack() as actx:
        EBP = int(os.environ.get("K_EB_BUFS", "4"))
        ebpool = actx.enter_context(tc.tile_pool(name="eb", bufs=EBP))
        lpsum = actx.enter_context(tc.tile_pool(name="lpsum", bufs=LPS_BUFS, space="PSUM"))
        apsum = actx.enter_context(tc.tile_pool(name="apsum", bufs=2, space="PSUM"))
        PPOOL_BUFS = int(os.environ.get("K_PPOOL_BUFS", "36"))
        ppool = actx.enter_context(tc.tile_pool(name="pT", bufs=PPOOL_BUFS))
        NP = int(os.environ.get("K_NORM_BUFS", "6"))
        npool = actx.enter_context(tc.tile_pool(name="norm", bufs=NP))
        opsum = actx.enter_context(tc.tile_pool(name="opsum", bufs=OPS_BUFS, space="PSUM"))
        tpsum = actx.enter_context(tc.tile_pool(name="tpsum", bufs=1, space="PSUM"))
        opool = actx.enter_context(tc.tile_pool(name="osb", bufs=3))

        def emit_pv(h2, st, avb, s_t, kt):
            # transposed PV: av[sq-tile, 64] += pT_slice.T @ v;  denominator
            # s[sq] += pT_slice.T @ ones into the shared s bank.
            # start/stop are bank-level: start=True clears has_written for the
            # whole bank, so only the first/last matmul into each bank carry
            # them; untouched elements store (not accumulate).
            b, n_p, hp_p, pT_p = st
            h = 2 * hp_p + h2
            for q in range(4):
                nc.tensor.matmul(
                    avb[:, h2, q, :],
                    lhsT=pT_p[(b, kt, h2)][:, 128 * q:128 * (q + 1)],
                    rhs=v_sb[b][kt][:, h, :],
                    start=(kt == 0 and h2 == 0 and q == 0),
                    stop=(kt == MS - 1 and h2 == 1 and q == 3),
                    skip_group_check=True,
                )
                si_ = 4 * h2 + q
                nc.tensor.matmul(
                    s_t[:, si_:si_ + 1],
                    lhsT=pT_p[(b, kt, h2)][:, 128 * q:128 * (q + 1)],
                    rhs=ones_sb[:],
                    start=(kt == 0 and h2 == 0 and q == 0),
                    stop=(kt == MS - 1 and h2 == 1 and q == 3),
                    skip_group_check=True,
                )

        def finalize_pv(h2, st, avb, s_t):
            # per-partition normalization (r = 1/denominator), then PE
            # transpose back to the attnT layout the O projection needs.
            b, n_p, hp_p, _ = st
            tp = tpsum.tile([64, 4, 128], BF16, tag="tp", name="tp")
            for q in range(4):
                si_ = 4 * h2 + q
                rcp = npool.tile([128, 1], F32, tag="rcp", name="rcp")
                nc.vector.reciprocal(rcp[:], s_t[:, si_:si_ + 1])
                attn_n = npool.tile([128, 64], BF16, tag="an", name="an")
                nc.vector.tensor_scalar(
                    attn_n[:], avb[:, h2, q, :], rcp[:], None, op0=ALU.mult,
                )
                nc.tensor.transpose(tp[:, q, :], attn_n[:], idn_sb[:])
                nc.vector.tensor_copy(
                    pair_sb[b][hp_p][64 * h2:64 * (h2 + 1),
                                     512 * n_p + 128 * q:512 * n_p + 128 * (q + 1)],
                    tp[:, q, :],
                )

        def emit_o_half(n, only_b=None):
            # O projection for sq half n (overlaps the next attention step)
            for b in ((only_b,) if only_b is not None else range(BC)):
                for mt in range(4 * n, 4 * (n + 1)):
                    o_t = opool.tile([128, DM], BF16, tag="osb", name="osb")
                    for no in range(DM // 512):
                        ps = opsum.tile([128, 512], F32, tag="o", name="o")
                        for kp in range(2):
                            nc.tensor.matmul(
                                ps[:],
                                lhsT=pair_sb[b][kp][:, 128 * mt:128 * (mt + 1)],
                                rhs=wo_sb[:, kp, 512 * no:512 * (no + 1)],
                                start=(kp == 0), stop=(kp == 1),
                            )
                        nc.vector.tensor_copy(o_t[:, 512 * no:512 * (no + 1)], ps[:])
                    nc.sync.dma_start(
                        out=out[b, 128 * mt:128 * (mt + 1), :],
                        in_=o_t[:],
                    )

        def emit_v_proj(vb):
            # deferred V projection: PE filler for the ACT-paced exp stream
            for mt in range(MS):
                vps = opsum.tile([128, DH], F32, tag="o", name="psv")
                for k in range(KT):
                    nc.tensor.matmul(
                        vps[:],
                        lhsT=y1g_all[vb][k // KG][:, k % KG,
                                                  128 * mt:128 * (mt + 1)],
                        rhs=wv_sb[:, k, :],
                        start=(k == 0), stop=(k == KT - 1),
                    )
                nc.vector.tensor_copy(
                    v_sb[vb][mt][:],
                    vps[:].rearrange("p (h d) -> p h d", d=DK),
                )

        def emit_qk_proj(w_sb, yg, dst, bias_sb, scale):
            # batch-1 Q/K projection, m-outer so only one PSUM group is in
            # flight (shares the opsum pool); DVE evacuation keeps the
            # scalar engine free for the exp stream.
            for m in range(2):
                for n2 in range(NQ):
                    ps = opsum.tile([128, 512], F32, tag="o", name="qk1")
                    for k in range(KT):
                        nc.tensor.matmul(
                            ps[:],
                            lhsT=w_sb[:, k, 128 * m:128 * (m + 1)],
                            rhs=yg[k // KG][:, k % KG, 512 * n2:512 * (n2 + 1)],
                            start=(k == 0), stop=(k == KT - 1),
                        )
                    nc.vector.tensor_scalar(
                        dst[m][:, 512 * n2:512 * (n2 + 1)], ps[:],
                        scale, bias_sb[:, m:m + 1], op0=ALU.mult, op1=ALU.add,
                    )

        # b-major steps: batch-0's ACT-bound attention window is filled with
        # batch-1's projections; PV of step i-1 interleaves with step i's QK.
        steps = [(b, n, hp) for b in range(BC) for n in range(NQ) for hp in range(2)]

        def filler0():
            y2g_all[1] = load_y(y2T, ybf, 1, "ybf")
            emit_v_proj(0)

        def filler1():
            y1g_all[1] = load_y(y1T, ybf1, 1, "ybf1")
            emit_qk_proj(wq_sb, y2g_all[1], qT_sb[1], bq_sb, SCALE)

        fillers = {0: filler0,
                   1: filler1,
                   2: lambda: emit_qk_proj(wk_sb, y1g_all[1], kT_sb[1], bk_sb, 1.0),
                   3: lambda: emit_v_proj(1)}
        prev = None            # (b, n, hp, pT) of the previous step
        for si, step in enumerate(steps + [None]):
            if step is not None:
                b, n, hp = step
                # exp(bias) tiles for this step's two heads
                eb_t = {}
                for h2 in range(2):
                    ebf = ebpool.tile([128, MS, 512], BF16, tag="eb", name="eb")
                    nc.sync.dma_start(
                        out=ebf[:],
                        in_=ebT[2 * hp + h2, :, 512 * n:512 * (n + 1)]
                        .rearrange("(k p) q -> p k q", p=128),
                    )
                    eb_t[h2] = ebf
                pT = {}
                for kt in range(MS):
                    for h2 in range(2):
                        lps = lpsum.tile([128, 512], F32, tag="l", name="l")
                        nc.tensor.matmul(
                            lps[:],
                            lhsT=kT_sb[b][hp][64 * h2:64 * (h2 + 1),
                                              128 * kt:128 * (kt + 1)],
                            rhs=qT_sb[b][hp][64 * h2:64 * (h2 + 1),
                                             512 * n:512 * (n + 1)],
                            start=True, stop=True,
                        )
                        pt = ppool.tile([128, 512], BF16, tag="pT", name="pT")
                        nc.scalar.activation(pt[:], lps[:], AF.Exp)
                        # bias enters multiplicatively: p~ = exp(l)*exp(b).
                        # bf16 SBUF tensor_tensor runs in the DVE 2x mode; a
                        # slice of the work goes to the idle gpsimd.
                        eng = nc.gpsimd if kt in (3, 6) else nc.vector
                        eng.tensor_tensor(
                            pt[:], pt[:], eb_t[h2][:, kt, :], op=ALU.mult,
                        )
                        pT[(b, kt, h2)] = pt
                    if prev is not None:
                        for h2 in range(2):   # previous step's PV, same kt
                            emit_pv(h2, prev, av_t, s_t, kt)
                if si in fillers:
                    fillers[si]()
            elif prev is not None:
                # tail: PV for the final step (no next step to interleave with)
                b_p = prev[0]
                for kt in range(MS):
                    for h2 in range(2):
                        emit_pv(h2, prev, av_t, s_t, kt)
            if prev is not None:
                b_p, n_p, hp_p, _ = prev
                for h2 in range(2):
                    finalize_pv(h2, prev, av_t, s_t)
                if hp_p == 1:
                    emit_o_half(n_p, only_b=b_p)
            if step is not None:
                prev = (b, n, hp, pT)
                av_t = apsum.tile([128, 2, 4, DK], F32, tag="av", name="av")
                s_t = tpsum.tile([128, 8], F32, tag="s", name="s")
            else:
                prev = None

# ====================== host wrapper ======================

def _prep_core_inputs(c, y1, y2, attn_bias, Wq, bq, Wk, bk, Wv, bv, Wo, bo):
    bp, hq = c // 4, c % 4
    bsl = slice(2 * bp, 2 * bp + 2)
    hsl = slice(DH * hq, DH * (hq + 1))
    bf16 = ml_dtypes.bfloat16
    f32 = np.float32
    bias_c = np.exp(attn_bias[0, 4 * hq:4 * hq + 4].astype(f32))
    return {
        "y1T": np.ascontiguousarray(y1[bsl].transpose(0, 2, 1)).astype(bf16),
        "y2T": np.ascontiguousarray(y2[bsl].transpose(0, 2, 1)).astype(bf16),
        "ebT": np.ascontiguousarray(bias_c.transpose(0, 2, 1)).astype(bf16),
        "idn": np.eye(128, dtype=f32).astype(bf16),
        "wq": np.ascontiguousarray(Wq[:, hsl]).astype(bf16),
        "wk": np.ascontiguousarray(Wk[:, hsl]).astype(bf16),
        "wv": np.ascontiguousarray(Wv[:, hsl]).astype(bf16),
        "wo": np.ascontiguousarray(Wo[hsl, :]).astype(bf16),
        "bqv": np.ascontiguousarray(
            (bq[hsl].astype(f32) * SCALE).reshape(2, 128).T
        ),
        "bkv": np.ascontiguousarray(bk[hsl].astype(f32).reshape(2, 128).T),
    }


def kernel(y1, y2, attn_bias, Wq, bq, Wk, bk, Wv, bv, Wo, bo):
    global _PROGRAM, LAST_RESULTS
    args = [np.asarray(x) for x in
            (y1, y2, attn_bias, Wq, bq, Wk, bk, Wv, bv, Wo, bo)]
    if _PROGRAM is None:
        _PROGRAM = build_program()
    nc, out_name = _PROGRAM

    in_maps = [_prep_core_inputs(c, *args) for c in range(N_CORES)]
    res = run_bass_kernel_spmd(nc, in_maps, list(range(N_CORES)), trace=TRACE)
    LAST_RESULTS = res

    out = np.zeros((B, Sq, DM), np.float32)
    for c in range(N_CORES):
        part = np.asarray(res.results[c][out_name]).astype(np.float32)
        bp = c // 4
        out[2 * bp] += part[0]
        out[2 * bp + 1] += part[1]
    # bias after partial-sum: bo plus the folded v-bias contribution bv @ Wo
    bias_full = (args[10].astype(np.float32)
                 + args[8].astype(np.float32) @ args[9].astype(np.float32))
    out += bias_full[None, None, :]
    return out



# revision 29
# speedup vs baseline: 1.0624x; 1.0624x over previous
"""
CrossAttention kernel for 8x Trainium2 NeuronCores (Bass/Tile).

Problem: nn_CrossAttention (B=4, Sq=Skv=1024, DM=1024, H=16, DK=64), fp32 I/O.

Sharding (Megatron-style hybrid, 8 cores = 2 batch-pairs x 4 head-quarters):
  core c -> batches {2*(c//4), 2*(c//4)+1}, heads {4*(c%4) .. 4*(c%4)+3}.
Each core computes Q/K/V projections for its 4 heads (256 of the 1024
Wq/Wk/Wv output dims), attention for those heads, and a partial O
projection (its 256 rows of Wo).  The host sums the 4 head-quarter
partials per batch and adds bo + bv@Wo (bias-after-allreduce; bv is
dropped on device since softmax weights sum to 1).

Device-side dataflow (per core), all matmuls bf16 with fp32 PSUM accumulate:
  - Host pre-transposes y1/y2 (feature-major yT, cast bf16) and ships
    ebT = exp(attn_bias)^T bf16 [HC, Skv, Sq]; no on-device casts needed.
  - qT/kT produced feature-major [dk, Sq]; bq (pre-scaled by 1/sqrt(dk)) and
    bk fused into the PSUM->SBUF evacuation via ACT activation bias.
  - v produced token-major [Skv, dk]; a ones-column is appended so the PV
    matmul emits the softmax denominator as PSUM row 64 for free (M=65).
  - logits computed transposed l^T [Skv, Sq] = k^T q; exp on ACT straight to
    bf16 in SBUF (no max-subtraction: |logits| <= ~15 is exact-exp-safe);
    the bias enters multiplicatively: p~ = exp(l) * exp(bias) via a DVE
    scalar_tensor_tensor multiply (4x mode: all-SBUF packed bf16).
  - attn^T = v^T p~^T (unnormalized) + denominator row; normalization applied
    at PSUM evacuation: r = 1/s broadcast across partitions via a DRAM
    round-trip DMA (0-step partition AP), DVE multiply during evacuation.
  - O projection token-major from assembled attn^T head-pair tiles.
"""

import os
import sys

sys.path.insert(0, "/opt/trn_rl_repo")

from contextlib import ExitStack

import numpy as np
import ml_dtypes

import concourse.bass as bass
import concourse.mybir as mybir
import concourse.tile as tile
from concourse import bacc
from concourse.bass_utils import run_bass_kernel_spmd

BF16 = mybir.dt.bfloat16
F32 = mybir.dt.float32
FP16 = mybir.dt.float16
AF = mybir.ActivationFunctionType
ALU = mybir.AluOpType

B, Sq, Skv, DM = 4, 1024, 1024, 1024
H, DK = 16, 64
HC = 4            # heads per core
DH = HC * DK      # head dims per core (256)
BC = 2            # batches per core
SCALE = DK ** -0.5
N_CORES = 8

_PROGRAM = None   # cached (nc, out_name)
TRACE = bool(int(os.environ.get("BASS_KERNEL_TRACE", "0")))
LPS_BUFS = int(os.environ.get("K_LPS_BUFS", "2"))
OPS_BUFS = int(os.environ.get("K_OPS_BUFS", "2"))
TT_POOL = int(os.environ.get("K_TT_POOL", "4"))      # exp-mults per step on gpsimd (of 16)
OEVAC_POOL = int(os.environ.get("K_OEVAC_POOL", "0"))  # 1: alternate DVE/Pool O evac
VEVAC_POOL = int(os.environ.get("K_VEVAC_POOL", "0"))  # V-proj evac on gpsimd
QKEVAC_POOL = int(os.environ.get("K_QKEVAC_POOL", "0"))  # b1 qk evac on gpsimd
FINCP_POOL = int(os.environ.get("K_FINCP_POOL", "0"))   # pair-copy on gpsimd
LAST_RESULTS = None


def build_program():
    """Build the per-core SPMD Bass program (identical on all 8 cores)."""
    nc = bacc.Bacc(
        "TRN2",
        target_bir_lowering=False,
        debug=False,
        num_devices=N_CORES,
    )

    # ---- DRAM parameters (per-core shards, host-prepared) ----
    y1T = nc.dram_tensor("y1T", [BC, DM, Skv], BF16, kind="ExternalInput")
    y2T = nc.dram_tensor("y2T", [BC, DM, Sq], BF16, kind="ExternalInput")
    ebT = nc.dram_tensor("ebT", [HC, Skv, Sq], BF16, kind="ExternalInput")  # biasT bf16
    idn = nc.dram_tensor("idn", [128, 128], BF16, kind="ExternalInput")  # identity
    wq = nc.dram_tensor("wq", [DM, DH], BF16, kind="ExternalInput")
    wk = nc.dram_tensor("wk", [DM, DH], BF16, kind="ExternalInput")
    wv = nc.dram_tensor("wv", [DM, DH], BF16, kind="ExternalInput")
    wo = nc.dram_tensor("wo", [DH, DM], BF16, kind="ExternalInput")
    bqv = nc.dram_tensor("bqv", [128, 2], F32, kind="ExternalInput")   # bq*SCALE, col-chunked
    bkv = nc.dram_tensor("bkv", [128, 2], F32, kind="ExternalInput")   # bk, col-chunked
    out = nc.dram_tensor("out", [BC, Sq, DM], BF16, kind="ExternalOutput")

    with tile.TileContext(nc) as tc, ExitStack() as ctx:
        build_kernel(ctx, tc, y1T, y2T, ebT, wq, wk, wv, wo, bqv, bkv, idn, out)

    nc.compile()
    return nc, "out"


def build_kernel(ctx, tc, y1T, y2T, ebT, wq, wk, wv, wo, bqv, bkv, idn, out):
    nc = tc.nc
    KT = DM // 128            # 8 contraction tiles for projections
    KG = 2                    # k-tiles per y DMA group
    NG = KT // KG             # 4 groups
    NQ = Sq // 512            # 2 moving-dim halves
    MS = Skv // 128           # 8 skv row tiles

    # ---------------- constant / weight loads ----------------
    consts = ctx.enter_context(tc.tile_pool(name="consts", bufs=1))

    wq_sb = consts.tile([128, KT, DH], BF16, tag="wq", name="wq")
    wk_sb = consts.tile([128, KT, DH], BF16, tag="wk", name="wk")
    wv_sb = consts.tile([128, KT, DH], BF16, tag="wv", name="wv")
    wo_sb = consts.tile([128, 2, DM], BF16, tag="wo", name="wo")   # [256,1024] -> 2 k-tiles
    bq_sb = consts.tile([128, 2], F32, tag="bq", name="bq")
    bk_sb = consts.tile([128, 2], F32, tag="bk", name="bk")
    idn_sb = consts.tile([128, 128], BF16, tag="idn", name="idn")
    ones_sb = consts.tile([128, 1], BF16, tag="ones", name="ones")
    nc.vector.memset(ones_sb[:], 1.0)

    # wq/bq issued first so the Q projection can start ASAP (wq in two chunks
    # for finer arrival granularity); the rest of the weight loads are
    # interleaved into the phase-P DMA stream (HWDGE is a serialized
    # resource, so issue order = arrival order).
    wq_r = wq.ap().rearrange("(k p) m -> p k m", p=128)
    nc.sync.dma_start(out=wq_sb[:, 0:1, :], in_=wq_r[:, 0:1, :])

    def load_v_weight():
        nc.sync.dma_start(out=wv_sb[:], in_=wv.ap().rearrange("(k p) m -> p k m", p=128))

    def load_o_weight():
        nc.sync.dma_start(out=wo_sb[:], in_=wo.ap().rearrange("(k p) m -> p k m", p=128))

    def load_idn():
        nc.sync.dma_start(out=idn_sb[:], in_=idn[:, :])

    def load_k_weights():
        nc.sync.dma_start(out=wk_sb[:], in_=wk.ap().rearrange("(k p) m -> p k m", p=128))
        nc.sync.dma_start(out=bk_sb[:], in_=bkv[:, :])

    # persistent activations (both batches)
    acts = ctx.enter_context(tc.tile_pool(name="acts", bufs=1))
    qT_sb = [[acts.tile([128, Sq], BF16, tag=f"qT{b}{p}", name=f"qT{b}{p}") for p in range(2)]
             for b in range(BC)]                       # [b][head-pair] rows=2x64 dk
    kT_sb = [[acts.tile([128, Skv], BF16, tag=f"kT{b}{p}", name=f"kT{b}{p}") for p in range(2)]
             for b in range(BC)]
    v_sb = [[acts.tile([128, HC, DK], BF16, tag=f"v{b}{m}", name=f"v{b}{m}") for m in range(MS)]
            for b in range(BC)]                        # [b][skv-tile][head][dk]
    pair_sb = [[acts.tile([128, Sq], BF16, tag=f"at{b}{p}", name=f"at{b}{p}") for p in range(2)]
               for b in range(BC)]                     # attnT head-pair tiles

    # ---------------- Phase P: batch-0 Q/K projections ----------------
    # Batch-1's Q/K (and both V projections) are deferred into batch-0's
    # attention steps, where the ACT-paced exp stream leaves the PE idle.
    y1g_all = {}
    y2g_all = {}
    # ybf1 fully double-buffered: y1 batch-1 loads must not carry WAR waits
    # (a waiting DMA at the SP queue head blocks every later DMA).
    ybf1 = ctx.enter_context(tc.tile_pool(name="ybf1", bufs=2 * NG))
    ybf = ctx.enter_context(tc.tile_pool(name="ybf", bufs=int(os.environ.get("K_Y_BUFS", "6"))))
    def load_y(dram, pool, b, tag, split_first=False, specs=None):
        # split_first: first contraction group loaded as two single-k DMAs so
        # the first projection matmul can start ~1.4us earlier at startup.
        if specs is None:
            specs = ([(0, 1), (1, 1)] + [(k, KG) for k in range(KG, KT, KG)]
                     ) if split_first else [(k, KG) for k in range(0, KT, KG)]
        gs = []
        for kbase, kcnt in specs:
            t = pool.tile([128, kcnt, Sq], BF16, tag=tag, name=tag)
            nc.sync.dma_start(
                out=t[:],
                in_=dram[b, 128 * kbase:128 * (kbase + kcnt), :]
                .rearrange("(k p) q -> p k q", p=128),
            )
            gs.append((t, kbase, kcnt))
        return gs

    def yap(gs, k):
        """[128, Sq] AP for contraction tile k of a load_y result."""
        for t, kb, kc in gs:
            if kb <= k < kb + kc:
                return t[:, k - kb, :]
        raise KeyError(k)

    with ExitStack() as pctx:
        ppsum = pctx.enter_context(tc.tile_pool(name="ppsum", bufs=4, space="PSUM"))

        # DMA issue order (serialized HWDGE/DMA pipe): wq k0, y2b0 k0/k1,
        # wq rest + bq, y2b0 rest, wk, y1b0, wv -- the first Q matmul can
        # start after ~330 KB instead of ~2.6 MB.
        y2g_all[0] = load_y(y2T, ybf, 0, "ybf", specs=[(0, 1)])
        nc.sync.dma_start(out=wq_sb[:, 1:KT, :], in_=wq_r[:, 1:KT, :])
        nc.sync.dma_start(out=bq_sb[:], in_=bqv[:, :])
        y2g_all[0] += load_y(y2T, ybf, 0, "ybf",
                             specs=[(1, 1)] + [(k, KG) for k in range(KG, KT, KG)])
        load_k_weights()
        qps = {(m, n): ppsum.tile([128, 512], F32, tag="ps", name="ps")
               for m in range(2) for n in range(NQ)}
        for k in range(KT):           # k-outer: start as soon as group lands
            for m in range(2):
                for n in range(NQ):
                    nc.tensor.matmul(
                        qps[(m, n)][:],
                        lhsT=wq_sb[:, k, 128 * m:128 * (m + 1)],
                        rhs=yap(y2g_all[0], k)[:, 512 * n:512 * (n + 1)],
                        start=(k == 0), stop=(k == KT - 1),
                    )
        for m in range(2):
            for n in range(NQ):
                nc.scalar.activation(
                    qT_sb[0][m][:, 512 * n:512 * (n + 1)], qps[(m, n)][:],
                    AF.Identity, bias=bq_sb[:, m:m + 1], scale=SCALE,
                )
        y1g_all[0] = load_y(y1T, ybf1, 0, "ybf1", split_first=True)
        load_v_weight()
        kps = {(m, n): ppsum.tile([128, 512], F32, tag="ps", name="ps")
               for m in range(2) for n in range(NQ)}
        for k in range(KT):
            for m in range(2):
                for n in range(NQ):
                    nc.tensor.matmul(
                        kps[(m, n)][:],
                        lhsT=wk_sb[:, k, 128 * m:128 * (m + 1)],
                        rhs=yap(y1g_all[0], k)[:, 512 * n:512 * (n + 1)],
                        start=(k == 0), stop=(k == KT - 1),
                    )
        for m in range(2):
            for n in range(NQ):
                nc.scalar.activation(
                    kT_sb[0][m][:, 512 * n:512 * (n + 1)], kps[(m, n)][:],
                    AF.Identity, bias=bk_sb[:, m:m + 1], scale=1.0,
                )
        # batch-1 y loads are issued inside the attention phase (steps 0-1)
        # to keep them out of the startup DMA flood

    # ---------------- Phase A: attention + interleaved O projection ----------------
    with ExitStack() as actx:
        EBP = int(os.environ.get("K_EB_BUFS", "6"))
        ebpool = actx.enter_context(tc.tile_pool(name="eb", bufs=EBP))
        lpsum = actx.enter_context(tc.tile_pool(name="lpsum", bufs=LPS_BUFS, space="PSUM"))
        apsum = actx.enter_context(tc.tile_pool(name="apsum", bufs=2, space="PSUM"))
        PPOOL_BUFS = int(os.environ.get("K_PPOOL_BUFS", "36"))
        ppool = actx.enter_context(tc.tile_pool(name="pT", bufs=PPOOL_BUFS))
        NP = int(os.environ.get("K_NORM_BUFS", "6"))
        npool = actx.enter_context(tc.tile_pool(name="norm", bufs=NP))
        opsum = actx.enter_context(tc.tile_pool(name="opsum", bufs=OPS_BUFS, space="PSUM"))
        tpsum = actx.enter_context(tc.tile_pool(name="tpsum", bufs=1, space="PSUM"))
        opool = actx.enter_context(tc.tile_pool(name="osb", bufs=6))

        def emit_pv(h2, st, avb, s_t, kt):
            # transposed PV: av[sq-tile, 64] += pT_slice.T @ v;  denominator
            # s[sq] += pT_slice.T @ ones into the shared s bank.
            # start/stop are bank-level: start=True clears has_written for the
            # whole bank, so only the first/last matmul into each bank carry
            # them; untouched elements store (not accumulate).
            b, n_p, hp_p, pT_p = st
            h = 2 * hp_p + h2
            for q in range(4):
                nc.tensor.matmul(
                    avb[:, h2, q, :],
                    lhsT=pT_p[(b, kt, h2)][:, 128 * q:128 * (q + 1)],
                    rhs=v_sb[b][kt][:, h, :],
                    start=(kt == 0 and h2 == 0 and q == 0),
                    stop=(kt == MS - 1 and h2 == 1 and q == 3),
                    skip_group_check=True,
                )
                si_ = 4 * h2 + q
                nc.tensor.matmul(
                    s_t[:, si_:si_ + 1],
                    lhsT=pT_p[(b, kt, h2)][:, 128 * q:128 * (q + 1)],
                    rhs=ones_sb[:],
                    start=(kt == 0 and h2 == 0 and q == 0),
                    stop=(kt == MS - 1 and h2 == 1 and q == 3),
                    skip_group_check=True,
                )

        # Deferred-PE-work queue: finalize / O-projection thunks for step
        # si-2 are drained inside step si's kt loop so the PE always has
        # independent work between the ACT-paced logits tiles and the
        # scalar/vector engines never drain at step boundaries.
        pe_defer = []

        def drain_defer(kt):
            remaining = MS - kt
            ndrain = (len(pe_defer) + remaining - 1) // remaining
            for _ in range(ndrain):
                if pe_defer:
                    pe_defer.pop(0)()

        def make_finalize_thunks(st, avb, rcp):
            # per-partition normalization (r = 1/denominator computed inline
            # at the step boundary), then PE transpose back to the attnT
            # layout the O projection needs.
            b, n_p, hp_p, _ = st
            tp = [None]

            def mk(h2, q):
                def thunk():
                    if tp[0] is None:
                        tp[0] = tpsum.tile([64, 2, 4, 128], BF16, tag="tp", name="tp")
                    si_ = 4 * h2 + q
                    attn_n = npool.tile([128, 64], BF16, tag="an", name="an")
                    nc.vector.tensor_scalar(
                        attn_n[:], avb[:, h2, q, :], rcp[:, si_:si_ + 1],
                        None, op0=ALU.mult,
                    )
                    nc.tensor.transpose(tp[0][:, h2, q, :], attn_n[:], idn_sb[:])
                    eng = nc.gpsimd if FINCP_POOL and (q % 2) else nc.vector
                    eng.tensor_copy(
                        pair_sb[b][hp_p][64 * h2:64 * (h2 + 1),
                                         512 * n_p + 128 * q:512 * n_p + 128 * (q + 1)],
                        tp[0][:, h2, q, :],
                    )
                return thunk

            # q-major: the O matmul for column-block q depends on BOTH h2
            # halves of that q-block, i.e. exactly the preceding two thunks.
            return [mk(h2, q) for q in range(4) for h2 in range(2)]

        def make_o_thunks(n, b):
            # O projection for sq half n.  The out-store dma_start is a
            # SEPARATE thunk scheduled after all four compute thunks: by the
            # time it issues, its o_t is (nearly) ready, so it does not park
            # at the head of the in-order SP DMA queue blocking later loads.
            o_ts = {}

            def mk(mt):
                def thunk():
                    o_t = opool.tile([128, DM], BF16, tag="osb", name="osb")
                    o_ts[mt] = o_t
                    for no in range(DM // 512):
                        ps = opsum.tile([128, 512], F32, tag="o", name="o")
                        for kp in range(2):
                            nc.tensor.matmul(
                                ps[:],
                                lhsT=pair_sb[b][kp][:, 128 * mt:128 * (mt + 1)],
                                rhs=wo_sb[:, kp, 512 * no:512 * (no + 1)],
                                start=(kp == 0), stop=(kp == 1),
                            )
                        eng = nc.gpsimd if OEVAC_POOL and (no + mt) % 2 else nc.vector
                        eng.tensor_copy(o_t[:, 512 * no:512 * (no + 1)], ps[:])
                return thunk

            def mk_dma(mt):
                def thunk():
                    nc.sync.dma_start(
                        out=out[b, 128 * mt:128 * (mt + 1), :],
                        in_=o_ts[mt][:],
                    )
                return thunk

            mts = list(range(4 * n, 4 * (n + 1)))
            return [mk(mt) for mt in mts], [mk_dma(mt) for mt in mts]

        def emit_v_proj(vb):
            # deferred V projection: PE filler for the ACT-paced exp stream
            for mt in range(MS):
                vps = opsum.tile([128, DH], F32, tag="o", name="psv")
                for k in range(KT):
                    nc.tensor.matmul(
                        vps[:],
                        lhsT=yap(y1g_all[vb], k)[:, 128 * mt:128 * (mt + 1)],
                        rhs=wv_sb[:, k, :],
                        start=(k == 0), stop=(k == KT - 1),
                    )
                eng = nc.gpsimd if VEVAC_POOL else nc.vector
                eng.tensor_copy(
                    v_sb[vb][mt][:],
                    vps[:].rearrange("p (h d) -> p h d", d=DK),
                )

        def emit_qk_proj(w_sb, yg, dst, bias_sb, scale):
            # batch-1 Q/K projection, m-outer so only one PSUM group is in
            # flight (shares the opsum pool); DVE/gpsimd evacuation keeps the
            # scalar engine free for the exp stream.
            for m in range(2):
                for n2 in range(NQ):
                    ps = opsum.tile([128, 512], F32, tag="o", name="qk1")
                    for k in range(KT):
                        nc.tensor.matmul(
                            ps[:],
                            lhsT=w_sb[:, k, 128 * m:128 * (m + 1)],
                            rhs=yap(yg, k)[:, 512 * n2:512 * (n2 + 1)],
                            start=(k == 0), stop=(k == KT - 1),
                        )
                    eng = nc.gpsimd if QKEVAC_POOL else nc.vector
                    eng.tensor_scalar(
                        dst[m][:, 512 * n2:512 * (n2 + 1)], ps[:],
                        scale, bias_sb[:, m:m + 1], op0=ALU.mult, op1=ALU.add,
                    )

        # Step order (n, b, hp): consecutive b-steps share the same (n, hp)
        # exp-bias tiles, halving ebT DMA traffic.  batch-1's Q/K/V
        # projections are fillers in steps 0-2 (emitted before the first b1
        # step so its logits find the tiles written).
        steps = [(b, n, hp) for n in range(NQ) for b in range(BC) for hp in range(2)]

        def filler0():
            y2g_all[1] = load_y(y2T, ybf, 1, "ybf")
            emit_v_proj(0)
            load_idn()
            y1g_all[1] = load_y(y1T, ybf1, 1, "ybf1")
            load_o_weight()

        def filler1():
            emit_qk_proj(wq_sb, y2g_all[1], qT_sb[1], bq_sb, SCALE)
            emit_qk_proj(wk_sb, y1g_all[1], kT_sb[1], bk_sb, 1.0)

        fillers = {0: filler0,
                   1: filler1,
                   2: lambda: emit_v_proj(1)}
        eb_cache = {}          # (n, hp) -> {h2: tile}
        EB_PREFETCH = {2: [(1, 0)], 3: [(1, 1)]}

        def load_eb_pair(n_l, hp_l):
            if (n_l, hp_l) in eb_cache:
                return
            eb_l = {}
            for h2 in range(2):
                ebf = ebpool.tile([128, MS, 512], BF16, tag="eb", name="eb")
                for kh in range(2):
                    nc.sync.dma_start(
                        out=ebf[:, 4 * kh:4 * (kh + 1), :],
                        in_=ebT[2 * hp_l + h2,
                                512 * kh:512 * (kh + 1),
                                512 * n_l:512 * (n_l + 1)]
                        .rearrange("(k p) q -> p k q", p=128),
                    )
                eb_l[h2] = ebf
            eb_cache[(n_l, hp_l)] = eb_l
        prev = None            # (b, n, hp, pT) of the previous step
        for si, step in enumerate(steps + [None]):
            if step is not None:
                b, n, hp = step
                # exp(bias) tiles for this step's two heads (loaded in kt
                # halves for finer arrival; reused by the later batch).  Both
                # hp pairs of an n-group are issued together so the hp=1 tiles
                # aren't queued behind the batch-1 y loads; the n=1 pairs are
                # prefetched during steps 2-3 (see EB_PREFETCH below).
                for n_l, hp_l in [(n, 0), (n, 1)] + EB_PREFETCH.get(si, []):
                    load_eb_pair(n_l, hp_l)
                eb_t = eb_cache[(n, hp)]
                pT = {}
                for kt in range(MS):
                    for h2 in range(2):
                        lps = lpsum.tile([128, 512], F32, tag="l", name="l")
                        nc.tensor.matmul(
                            lps[:],
                            lhsT=kT_sb[b][hp][64 * h2:64 * (h2 + 1),
                                              128 * kt:128 * (kt + 1)],
                            rhs=qT_sb[b][hp][64 * h2:64 * (h2 + 1),
                                             512 * n:512 * (n + 1)],
                            start=True, stop=True,
                        )
                        pt = ppool.tile([128, 512], BF16, tag="pT", name="pT")
                        nc.scalar.activation(pt[:], lps[:], AF.Exp)
                        # bias enters multiplicatively: p~ = exp(l)*exp(b).
                        # bf16 SBUF tensor_tensor runs in the DVE 2x mode; a
                        # slice of the work goes to the idle gpsimd.
                        tt_i = 2 * kt + h2
                        eng = nc.gpsimd if (tt_i * TT_POOL) // 16 != ((tt_i + 1) * TT_POOL) // 16 else nc.vector
                        eng.tensor_tensor(
                            pt[:], pt[:], eb_t[h2][:, kt, :], op=ALU.mult,
                        )
                        pT[(b, kt, h2)] = pt
                    if prev is not None:
                        for h2 in range(2):   # previous step's PV, same kt
                            emit_pv(h2, prev, av_t, s_t, kt)
                    drain_defer(kt)
                if si in fillers:
                    fillers[si]()
            elif prev is not None:
                # tail: PV for the final step (no next step to interleave
                # with); drain any remaining deferred work between tiles.
                for kt in range(MS):
                    for h2 in range(2):
                        emit_pv(h2, prev, av_t, s_t, kt)
                    drain_defer(kt)
            if prev is not None:
                b_p, n_p, hp_p, _ = prev
                # denominator reciprocal inline (frees the single-buffered
                # s bank before the next step's PV writes it); the rest of
                # the finalize + O projection is deferred into the NEXT
                # step's kt loop.  Finalize and O thunks are interleaved so
                # consecutive opsum allocations are spaced apart (slack for
                # the DVE evacuation before the PSUM slot is reused).
                rcp = npool.tile([128, 8], F32, tag="rcp", name="rcp")
                nc.vector.reciprocal(rcp[:], s_t[:])
                fins = make_finalize_thunks(prev, av_t, rcp)
                if hp_p == 1:
                    omms, odmas = make_o_thunks(n_p, b_p)
                    for i in range(4):
                        pe_defer.extend(fins[2 * i:2 * i + 2])
                        pe_defer.append(omms[i])
                        if i >= 1:
                            pe_defer.append(odmas[i - 1])
                    pe_defer.append(odmas[3])
                else:
                    pe_defer.extend(fins)
            if step is not None:
                prev = (b, n, hp, pT)
                av_t = apsum.tile([128, 2, 4, DK], F32, tag="av", name="av")
                s_t = tpsum.tile([128, 8], F32, tag="s", name="s")
            else:
                prev = None
        # tail: flush any remaining deferred finalize / O-projection work
        while pe_defer:
            pe_defer.pop(0)()

# ====================== host wrapper ======================

def _prep_core_inputs(c, y1, y2, attn_bias, Wq, bq, Wk, bk, Wv, bv, Wo, bo):
    bp, hq = c // 4, c % 4
    bsl = slice(2 * bp, 2 * bp + 2)
    hsl = slice(DH * hq, DH * (hq + 1))
    bf16 = ml_dtypes.bfloat16
    f32 = np.float32
    bias_c = np.exp(attn_bias[0, 4 * hq:4 * hq + 4].astype(f32))
    return {
        "y1T": np.ascontiguousarray(y1[bsl].transpose(0, 2, 1)).astype(bf16),
        "y2T": np.ascontiguousarray(y2[bsl].transpose(0, 2, 1)).astype(bf16),
        "ebT": np.ascontiguousarray(bias_c.transpose(0, 2, 1)).astype(bf16),
        "idn": np.eye(128, dtype=f32).astype(bf16),
        "wq": np.ascontiguousarray(Wq[:, hsl]).astype(bf16),
        "wk": np.ascontiguousarray(Wk[:, hsl]).astype(bf16),
        "wv": np.ascontiguousarray(Wv[:, hsl]).astype(bf16),
        "wo": np.ascontiguousarray(Wo[hsl, :]).astype(bf16),
        "bqv": np.ascontiguousarray(
            (bq[hsl].astype(f32) * SCALE).reshape(2, 128).T
        ),
        "bkv": np.ascontiguousarray(bk[hsl].astype(f32).reshape(2, 128).T),
    }


def kernel(y1, y2, attn_bias, Wq, bq, Wk, bk, Wv, bv, Wo, bo):
    global _PROGRAM, LAST_RESULTS
    args = [np.asarray(x) for x in
            (y1, y2, attn_bias, Wq, bq, Wk, bk, Wv, bv, Wo, bo)]
    if _PROGRAM is None:
        _PROGRAM = build_program()
    nc, out_name = _PROGRAM

    in_maps = [_prep_core_inputs(c, *args) for c in range(N_CORES)]
    res = run_bass_kernel_spmd(nc, in_maps, list(range(N_CORES)), trace=TRACE)
    LAST_RESULTS = res

    out = np.zeros((B, Sq, DM), np.float32)
    for c in range(N_CORES):
        part = np.asarray(res.results[c][out_name]).astype(np.float32)
        bp = c // 4
        out[2 * bp] += part[0]
        out[2 * bp + 1] += part[1]
    # bias after partial-sum: bo plus the folded v-bias contribution bv @ Wo
    bias_full = (args[10].astype(np.float32)
                 + args[8].astype(np.float32) @ args[9].astype(np.float32))
    out += bias_full[None, None, :]
    return out



# revision 46
# speedup vs baseline: 1.0959x; 1.0316x over previous
"""
CrossAttention kernel for 8x Trainium2 NeuronCores (Bass/Tile).

Problem: nn_CrossAttention (B=4, Sq=Skv=1024, DM=1024, H=16, DK=64), fp32 I/O.

Sharding (Megatron-style hybrid, 8 cores = 2 batch-pairs x 4 head-quarters):
  core c -> batches {2*(c//4), 2*(c//4)+1}, heads {4*(c%4) .. 4*(c%4)+3}.
Each core computes Q/K/V projections for its 4 heads (256 of the 1024
Wq/Wk/Wv output dims), attention for those heads, and a partial O
projection (its 256 rows of Wo).  The host sums the 4 head-quarter
partials per batch and adds bo + bv@Wo (bias-after-allreduce; bv is
dropped on device since softmax weights sum to 1).

Device-side dataflow (per core), all matmuls bf16 with fp32 PSUM accumulate:
  - Host pre-transposes y1/y2 (feature-major yT, cast bf16) and ships
    ebT = exp(attn_bias)^T bf16 [HC, Skv, Sq]; no on-device casts needed.
  - qT/kT produced feature-major [dk, Sq]; bq (pre-scaled by 1/sqrt(dk)) and
    bk fused into the PSUM->SBUF evacuation via ACT activation bias.
  - v produced token-major [Skv, dk]; a ones-column is appended so the PV
    matmul emits the softmax denominator as PSUM row 64 for free (M=65).
  - logits computed transposed l^T [Skv, Sq] = k^T q; exp on ACT straight to
    bf16 in SBUF (no max-subtraction: |logits| <= ~15 is exact-exp-safe);
    the bias enters multiplicatively: p~ = exp(l) * exp(bias) via a DVE
    scalar_tensor_tensor multiply (4x mode: all-SBUF packed bf16).
  - attn^T = v^T p~^T (unnormalized) + denominator row; normalization applied
    at PSUM evacuation: r = 1/s broadcast across partitions via a DRAM
    round-trip DMA (0-step partition AP), DVE multiply during evacuation.
  - O projection token-major from assembled attn^T head-pair tiles.
"""

import os
import sys

sys.path.insert(0, "/opt/trn_rl_repo")

from contextlib import ExitStack

import numpy as np
import ml_dtypes

import concourse.bass as bass
import concourse.mybir as mybir
import concourse.tile as tile
from concourse import bacc
from concourse.bass_utils import run_bass_kernel_spmd

BF16 = mybir.dt.bfloat16
F32 = mybir.dt.float32
FP16 = mybir.dt.float16
AF = mybir.ActivationFunctionType
ALU = mybir.AluOpType

B, Sq, Skv, DM = 4, 1024, 1024, 1024
H, DK = 16, 64
HC = 4            # heads per core
DH = HC * DK      # head dims per core (256)
BC = 2            # batches per core
SCALE = DK ** -0.5
N_CORES = 8

_PROGRAM = None   # cached (nc, out_name)
TRACE = bool(int(os.environ.get("BASS_KERNEL_TRACE", "0")))
LPS_BUFS = int(os.environ.get("K_LPS_BUFS", "2"))
OPS_BUFS = int(os.environ.get("K_OPS_BUFS", "2"))
TT_POOL = int(os.environ.get("K_TT_POOL", "3"))      # exp-mults per step on gpsimd (of 16)
OEVAC_POOL = int(os.environ.get("K_OEVAC_POOL", "0"))  # 1: alternate DVE/Pool O evac
VEVAC_POOL = int(os.environ.get("K_VEVAC_POOL", "0"))  # V-proj evac on gpsimd
QKEVAC_POOL = int(os.environ.get("K_QKEVAC_POOL", "0"))  # b1 qk evac on gpsimd
FINCP_POOL = int(os.environ.get("K_FINCP_POOL", "0"))   # pair-copy on gpsimd
LAST_RESULTS = None


def build_program():
    """Build the per-core SPMD Bass program (identical on all 8 cores)."""
    nc = bacc.Bacc(
        "TRN2",
        target_bir_lowering=False,
        debug=False,
        num_devices=N_CORES,
    )

    # ---- DRAM parameters (per-core shards, host-prepared) ----
    y1T = nc.dram_tensor("y1T", [BC, DM, Skv], BF16, kind="ExternalInput")
    y2T = nc.dram_tensor("y2T", [BC, DM, Sq], BF16, kind="ExternalInput")
    ebT = nc.dram_tensor("ebT", [HC, Skv, Sq], BF16, kind="ExternalInput")  # biasT bf16
    idn = nc.dram_tensor("idn", [128, 128], BF16, kind="ExternalInput")  # identity
    wq = nc.dram_tensor("wq", [DM, DH], BF16, kind="ExternalInput")
    wk = nc.dram_tensor("wk", [DM, DH], BF16, kind="ExternalInput")
    wv = nc.dram_tensor("wv", [DM, DH], BF16, kind="ExternalInput")
    wo = nc.dram_tensor("wo", [DH, DM], BF16, kind="ExternalInput")
    bqv = nc.dram_tensor("bqv", [128, 2], F32, kind="ExternalInput")   # bq*SCALE, col-chunked
    bkv = nc.dram_tensor("bkv", [128, 2], F32, kind="ExternalInput")   # bk, col-chunked
    out = nc.dram_tensor("out", [BC, Sq, DM], BF16, kind="ExternalOutput")

    with tile.TileContext(nc) as tc, ExitStack() as ctx:
        build_kernel(ctx, tc, y1T, y2T, ebT, wq, wk, wv, wo, bqv, bkv, idn, out)

    nc.compile()
    return nc, "out"


def build_kernel(ctx, tc, y1T, y2T, ebT, wq, wk, wv, wo, bqv, bkv, idn, out):
    nc = tc.nc
    KT = DM // 128            # 8 contraction tiles for projections
    KG = 2                    # k-tiles per y DMA group
    NG = KT // KG             # 4 groups
    NQ = Sq // 512            # 2 moving-dim halves
    MS = Skv // 128           # 8 skv row tiles

    # ---------------- constant / weight loads ----------------
    consts = ctx.enter_context(tc.tile_pool(name="consts", bufs=1))

    wq_sb = consts.tile([128, KT, DH], BF16, tag="wq", name="wq")
    wk_sb = consts.tile([128, KT, DH], BF16, tag="wk", name="wk")
    wv_sb = consts.tile([128, KT, DH], BF16, tag="wv", name="wv")
    wo_sb = consts.tile([128, 2, DM], BF16, tag="wo", name="wo")   # [256,1024] -> 2 k-tiles
    bq_sb = consts.tile([128, 2], F32, tag="bq", name="bq")
    bk_sb = consts.tile([128, 2], F32, tag="bk", name="bk")
    idn_sb = consts.tile([128, 128], BF16, tag="idn", name="idn")
    ones_sb = consts.tile([128, 1], BF16, tag="ones", name="ones")
    nc.vector.memset(ones_sb[:], 1.0)

    # wq/bq issued first so the Q projection can start ASAP (wq in two chunks
    # for finer arrival granularity); the rest of the weight loads are
    # interleaved into the phase-P DMA stream (HWDGE is a serialized
    # resource, so issue order = arrival order).
    wq_r = wq.ap().rearrange("(k p) m -> p k m", p=128)
    nc.sync.dma_start(out=wq_sb[:, 0:1, :], in_=wq_r[:, 0:1, :])

    def load_v_weight():
        nc.sync.dma_start(out=wv_sb[:], in_=wv.ap().rearrange("(k p) m -> p k m", p=128))

    def load_o_weight():
        nc.sync.dma_start(out=wo_sb[:], in_=wo.ap().rearrange("(k p) m -> p k m", p=128))

    def load_idn():
        nc.sync.dma_start(out=idn_sb[:], in_=idn[:, :])

    def load_k_weights():
        nc.sync.dma_start(out=wk_sb[:], in_=wk.ap().rearrange("(k p) m -> p k m", p=128))
        nc.sync.dma_start(out=bk_sb[:], in_=bkv[:, :])

    # persistent activations (both batches)
    acts = ctx.enter_context(tc.tile_pool(name="acts", bufs=1))
    qT_sb = [[acts.tile([128, Sq], BF16, tag=f"qT{b}{p}", name=f"qT{b}{p}") for p in range(2)]
             for b in range(BC)]                       # [b][head-pair] rows=2x64 dk
    kT_sb = [[acts.tile([128, Skv], BF16, tag=f"kT{b}{p}", name=f"kT{b}{p}") for p in range(2)]
             for b in range(BC)]
    v_sb = [[acts.tile([128, HC, DK], BF16, tag=f"v{b}{m}", name=f"v{b}{m}") for m in range(MS)]
            for b in range(BC)]                        # [b][skv-tile][head][dk]
    pair_sb = [[acts.tile([128, Sq], BF16, tag=f"at{b}{p}", name=f"at{b}{p}") for p in range(2)]
               for b in range(BC)]                     # attnT head-pair tiles

    # ---------------- Phase P: batch-0 Q/K projections ----------------
    # Batch-1's Q/K (and both V projections) are deferred into batch-0's
    # attention steps, where the ACT-paced exp stream leaves the PE idle.
    y1g_all = {}
    y2g_all = {}
    # ybf1 fully double-buffered: y1 batch-1 loads must not carry WAR waits
    # (a waiting DMA at the SP queue head blocks every later DMA).
    ybf1 = ctx.enter_context(tc.tile_pool(name="ybf1", bufs=2 * NG))
    ybf = ctx.enter_context(tc.tile_pool(name="ybf", bufs=int(os.environ.get("K_Y_BUFS", "6"))))
    def load_y(dram, pool, b, tag, split_first=False, specs=None):
        # split_first: first contraction group loaded as two single-k DMAs so
        # the first projection matmul can start ~1.4us earlier at startup.
        if specs is None:
            specs = ([(0, 1), (1, 1)] + [(k, KG) for k in range(KG, KT, KG)]
                     ) if split_first else [(k, KG) for k in range(0, KT, KG)]
        gs = []
        for kbase, kcnt in specs:
            t = pool.tile([128, kcnt, Sq], BF16, tag=tag, name=tag)
            nc.sync.dma_start(
                out=t[:],
                in_=dram[b, 128 * kbase:128 * (kbase + kcnt), :]
                .rearrange("(k p) q -> p k q", p=128),
            )
            gs.append((t, kbase, kcnt))
        return gs

    def yap(gs, k):
        """[128, Sq] AP for contraction tile k of a load_y result."""
        for t, kb, kc in gs:
            if kb <= k < kb + kc:
                return t[:, k - kb, :]
        raise KeyError(k)

    with ExitStack() as pctx:
        ppsum = pctx.enter_context(tc.tile_pool(name="ppsum", bufs=4, space="PSUM"))

        # DMA issue order (serialized HWDGE/DMA pipe): wq k0, y2b0 k0/k1,
        # wq rest + bq, y2b0 rest, wk, y1b0, wv -- the first Q matmul can
        # start after ~330 KB instead of ~2.6 MB.
        y2g_all[0] = load_y(y2T, ybf, 0, "ybf", specs=[(0, 1)])
        nc.sync.dma_start(out=wq_sb[:, 1:KT, :], in_=wq_r[:, 1:KT, :])
        nc.sync.dma_start(out=bq_sb[:], in_=bqv[:, :])
        y2g_all[0] += load_y(y2T, ybf, 0, "ybf",
                             specs=[(1, 1)] + [(k, KG) for k in range(KG, KT, KG)])
        load_k_weights()
        qps = {(m, n): ppsum.tile([128, 512], F32, tag="ps", name="ps")
               for m in range(2) for n in range(NQ)}
        for k in range(KT):           # k-outer: start as soon as group lands
            for m in range(2):
                for n in range(NQ):
                    nc.tensor.matmul(
                        qps[(m, n)][:],
                        lhsT=wq_sb[:, k, 128 * m:128 * (m + 1)],
                        rhs=yap(y2g_all[0], k)[:, 512 * n:512 * (n + 1)],
                        start=(k == 0), stop=(k == KT - 1),
                    )
        for m in range(2):
            for n in range(NQ):
                nc.scalar.activation(
                    qT_sb[0][m][:, 512 * n:512 * (n + 1)], qps[(m, n)][:],
                    AF.Identity, bias=bq_sb[:, m:m + 1], scale=SCALE,
                )
        y1g_all[0] = load_y(y1T, ybf1, 0, "ybf1", split_first=True)
        load_v_weight()
        kps = {(m, n): ppsum.tile([128, 512], F32, tag="ps", name="ps")
               for m in range(2) for n in range(NQ)}
        for k in range(KT):
            for m in range(2):
                for n in range(NQ):
                    nc.tensor.matmul(
                        kps[(m, n)][:],
                        lhsT=wk_sb[:, k, 128 * m:128 * (m + 1)],
                        rhs=yap(y1g_all[0], k)[:, 512 * n:512 * (n + 1)],
                        start=(k == 0), stop=(k == KT - 1),
                    )
        for m in range(2):
            for n in range(NQ):
                nc.scalar.activation(
                    kT_sb[0][m][:, 512 * n:512 * (n + 1)], kps[(m, n)][:],
                    AF.Identity, bias=bk_sb[:, m:m + 1], scale=1.0,
                )
        # batch-1 y loads are issued inside the attention phase (steps 0-1)
        # to keep them out of the startup DMA flood

    # ---------------- Phase A: attention + interleaved O projection ----------------
    with ExitStack() as actx:
        EBP = int(os.environ.get("K_EB_BUFS", "6"))
        ebpool = actx.enter_context(tc.tile_pool(name="eb", bufs=EBP))
        lpsum = actx.enter_context(tc.tile_pool(name="lpsum", bufs=LPS_BUFS, space="PSUM"))
        apsum = actx.enter_context(tc.tile_pool(name="apsum", bufs=2, space="PSUM"))
        PPOOL_BUFS = int(os.environ.get("K_PPOOL_BUFS", "36"))
        ppool = actx.enter_context(tc.tile_pool(name="pT", bufs=PPOOL_BUFS))
        NP = int(os.environ.get("K_NORM_BUFS", "6"))
        npool = actx.enter_context(tc.tile_pool(name="norm", bufs=NP))
        opsum = actx.enter_context(tc.tile_pool(name="opsum", bufs=OPS_BUFS, space="PSUM"))
        tpsum = actx.enter_context(tc.tile_pool(name="tpsum", bufs=1, space="PSUM"))
        opool = actx.enter_context(tc.tile_pool(name="osb", bufs=int(os.environ.get("K_O_BUFS", "6"))))

        def emit_pv(h2, st, avb, s_t, kt):
            # transposed PV: av[sq-tile, 64] += pT_slice.T @ v;  denominator
            # s[sq] += pT_slice.T @ ones into the shared s bank.
            # start/stop are bank-level: start=True clears has_written for the
            # whole bank, so only the first/last matmul into each bank carry
            # them; untouched elements store (not accumulate).
            b, n_p, hp_p, pT_p = st
            h = 2 * hp_p + h2
            for q in range(4):
                nc.tensor.matmul(
                    avb[:, h2, q, :],
                    lhsT=pT_p[(b, kt, h2)][:, 128 * q:128 * (q + 1)],
                    rhs=v_sb[b][kt][:, h, :],
                    start=(kt == 0 and h2 == 0 and q == 0),
                    stop=(kt == MS - 1 and h2 == 1 and q == 3),
                    skip_group_check=True,
                )
                si_ = 4 * h2 + q
                nc.tensor.matmul(
                    s_t[:, si_:si_ + 1],
                    lhsT=pT_p[(b, kt, h2)][:, 128 * q:128 * (q + 1)],
                    rhs=ones_sb[:],
                    start=(kt == 0 and h2 == 0 and q == 0),
                    stop=(kt == MS - 1 and h2 == 1 and q == 3),
                    skip_group_check=True,
                )

        # Deferred-PE-work queue: finalize / O-projection thunks for step
        # si-2 are drained inside step si's kt loop so the PE always has
        # independent work between the ACT-paced logits tiles and the
        # scalar/vector engines never drain at step boundaries.
        pe_defer = []

        def drain_defer(kt):
            remaining = MS - kt
            ndrain = (len(pe_defer) + remaining - 1) // remaining
            for _ in range(ndrain):
                if pe_defer:
                    pe_defer.pop(0)()

        def make_finalize_thunks(st, avb, rcp):
            # per-partition normalization (r = 1/denominator computed inline
            # at the step boundary), then PE transpose back to the attnT
            # layout the O projection needs.
            b, n_p, hp_p, _ = st
            tp = [None]

            def mk(q):
                # both h2 halves staged side by side in one [128,128] block:
                # a single PE transpose + single copy lands the full pair
                # partition range for this q column-block.
                def thunk():
                    if tp[0] is None:
                        tp[0] = tpsum.tile([128, 4, 128], BF16, tag="tp", name="tp")
                    attn_n = npool.tile([128, 128], BF16, tag="an", name="an")
                    for h2 in range(2):
                        nc.vector.tensor_scalar(
                            attn_n[:, 64 * h2:64 * (h2 + 1)], avb[:, h2, q, :],
                            rcp[:, 4 * h2 + q:4 * h2 + q + 1],
                            None, op0=ALU.mult,
                        )
                    nc.tensor.transpose(tp[0][:, q, :], attn_n[:], idn_sb[:])
                    eng = nc.gpsimd if FINCP_POOL and (q % 2) else nc.vector
                    eng.tensor_copy(
                        pair_sb[b][hp_p][:, 512 * n_p + 128 * q:
                                         512 * n_p + 128 * (q + 1)],
                        tp[0][:, q, :],
                    )
                return thunk

            # q-major: the O matmul for column-block q depends on exactly the
            # preceding thunk.
            return [mk(q) for q in range(4)]

        def make_o_thunks(n, b):
            # O projection for sq half n.  The out-store dma_start is a
            # SEPARATE thunk scheduled after all four compute thunks: by the
            # time it issues, its o_t is (nearly) ready, so it does not park
            # at the head of the in-order SP DMA queue blocking later loads.
            o_ts = {}

            def mk(mt, no):
                def thunk():
                    if no == 0:
                        o_ts[mt] = opool.tile([128, DM], BF16, tag="osb", name="osb")
                    o_t = o_ts[mt]
                    ps = opsum.tile([128, 512], F32, tag="o", name="o")
                    for kp in range(2):
                        nc.tensor.matmul(
                            ps[:],
                            lhsT=pair_sb[b][kp][:, 128 * mt:128 * (mt + 1)],
                            rhs=wo_sb[:, kp, 512 * no:512 * (no + 1)],
                            start=(kp == 0), stop=(kp == 1),
                        )
                    i_e = (mt % 2) if int(os.environ.get("K_OEVAC_MT", "0")) else (no + 2 * mt) % 2
                    if OEVAC_POOL == 0:
                        eng = nc.vector
                    elif OEVAC_POOL == 1:
                        eng = nc.gpsimd if i_e else nc.vector
                    elif OEVAC_POOL == 2:
                        eng = nc.scalar if i_e else nc.vector
                    else:
                        eng = (nc.vector, nc.gpsimd, nc.scalar)[(no + 2 * mt) % 3]
                    if eng is nc.scalar:
                        eng.copy(o_t[:, 512 * no:512 * (no + 1)], ps[:])
                    else:
                        eng.tensor_copy(o_t[:, 512 * no:512 * (no + 1)], ps[:])
                return thunk

            def mk_dma(mt, no):
                def thunk():
                    nc.sync.dma_start(
                        out=out[b, 128 * mt:128 * (mt + 1),
                                512 * no:512 * (no + 1)],
                        in_=o_ts[mt][:, 512 * no:512 * (no + 1)],
                    )
                return thunk

            mts = list(range(4 * n, 4 * (n + 1)))
            return ([[mk(mt, 0), mk(mt, 1)] for mt in mts],
                    [[mk_dma(mt, 0), mk_dma(mt, 1)] for mt in mts])

        def emit_v_proj(vb):
            # deferred V projection: PE filler for the ACT-paced exp stream
            for mt in range(MS):
                vps = opsum.tile([128, DH], F32, tag="o", name="psv")
                for k in range(KT):
                    nc.tensor.matmul(
                        vps[:],
                        lhsT=yap(y1g_all[vb], k)[:, 128 * mt:128 * (mt + 1)],
                        rhs=wv_sb[:, k, :],
                        start=(k == 0), stop=(k == KT - 1),
                    )
                eng = nc.gpsimd if VEVAC_POOL else nc.vector
                eng.tensor_copy(
                    v_sb[vb][mt][:],
                    vps[:].rearrange("p (h d) -> p h d", d=DK),
                )

        def emit_qk_proj(w_sb, yg, dst, bias_sb, scale):
            # batch-1 Q/K projection, m-outer so only one PSUM group is in
            # flight (shares the opsum pool); DVE/gpsimd evacuation keeps the
            # scalar engine free for the exp stream.
            for m in range(2):
                for n2 in range(NQ):
                    ps = opsum.tile([128, 512], F32, tag="o", name="qk1")
                    for k in range(KT):
                        nc.tensor.matmul(
                            ps[:],
                            lhsT=w_sb[:, k, 128 * m:128 * (m + 1)],
                            rhs=yap(yg, k)[:, 512 * n2:512 * (n2 + 1)],
                            start=(k == 0), stop=(k == KT - 1),
                        )
                    eng = nc.gpsimd if QKEVAC_POOL else nc.vector
                    eng.tensor_scalar(
                        dst[m][:, 512 * n2:512 * (n2 + 1)], ps[:],
                        scale, bias_sb[:, m:m + 1], op0=ALU.mult, op1=ALU.add,
                    )

        # Step order (n, b, hp): consecutive b-steps share the same (n, hp)
        # exp-bias tiles, halving ebT DMA traffic.  batch-1's Q/K/V
        # projections are fillers in steps 0-2 (emitted before the first b1
        # step so its logits find the tiles written).
        steps = [(b, n, hp) for n in range(NQ) for b in range(BC) for hp in range(2)]

        def filler0():
            y2g_all[1] = load_y(y2T, ybf, 1, "ybf")
            emit_v_proj(0)
            load_idn()
            y1g_all[1] = load_y(y1T, ybf1, 1, "ybf1")
            load_o_weight()

        def filler1():
            emit_qk_proj(wq_sb, y2g_all[1], qT_sb[1], bq_sb, SCALE)
            emit_qk_proj(wk_sb, y1g_all[1], kT_sb[1], bk_sb, 1.0)

        fillers = {0: filler0,
                   1: filler1,
                   2: lambda: emit_v_proj(1)}
        eb_cache = {}          # (n, hp) -> {h2: tile}
        EB_PREFETCH = {2: [(1, 0)], 3: [(1, 1)]}

        def load_eb_pair(n_l, hp_l):
            if (n_l, hp_l) in eb_cache:
                return
            eb_l = {}
            for h2 in range(2):
                ebf = ebpool.tile([128, MS, 512], BF16, tag="eb", name="eb")
                for kh in range(2):
                    nc.sync.dma_start(
                        out=ebf[:, 4 * kh:4 * (kh + 1), :],
                        in_=ebT[2 * hp_l + h2,
                                512 * kh:512 * (kh + 1),
                                512 * n_l:512 * (n_l + 1)]
                        .rearrange("(k p) q -> p k q", p=128),
                    )
                eb_l[h2] = ebf
            eb_cache[(n_l, hp_l)] = eb_l
        prev = None            # (b, n, hp, pT) of the previous step
        for si, step in enumerate(steps + [None]):
            if step is not None:
                b, n, hp = step
                # exp(bias) tiles for this step's two heads (loaded in kt
                # halves for finer arrival; reused by the later batch).  Both
                # hp pairs of an n-group are issued together so the hp=1 tiles
                # aren't queued behind the batch-1 y loads; the n=1 pairs are
                # prefetched during steps 2-3 (see EB_PREFETCH below).
                for n_l, hp_l in [(n, 0), (n, 1)] + EB_PREFETCH.get(si, []):
                    load_eb_pair(n_l, hp_l)
                eb_t = eb_cache[(n, hp)]
                pT = {}
                for kt in range(MS):
                    for h2 in range(2):
                        lps = lpsum.tile([128, 512], F32, tag="l", name="l")
                        nc.tensor.matmul(
                            lps[:],
                            lhsT=kT_sb[b][hp][64 * h2:64 * (h2 + 1),
                                              128 * kt:128 * (kt + 1)],
                            rhs=qT_sb[b][hp][64 * h2:64 * (h2 + 1),
                                             512 * n:512 * (n + 1)],
                            start=True, stop=True,
                        )
                        pt = ppool.tile([128, 512], BF16, tag="pT", name="pT")
                        nc.scalar.activation(pt[:], lps[:], AF.Exp)
                        # bias enters multiplicatively: p~ = exp(l)*exp(b).
                        # bf16 SBUF tensor_tensor runs in the DVE 2x mode; a
                        # slice of the work goes to the idle gpsimd.
                        tt_i = 2 * kt + h2
                        eng = nc.gpsimd if (tt_i * TT_POOL) // 16 != ((tt_i + 1) * TT_POOL) // 16 else nc.vector
                        eng.tensor_tensor(
                            pt[:], pt[:], eb_t[h2][:, kt, :], op=ALU.mult,
                        )
                        pT[(b, kt, h2)] = pt
                    if prev is not None:
                        for h2 in range(2):   # previous step's PV, same kt
                            emit_pv(h2, prev, av_t, s_t, kt)
                    drain_defer(kt)
                if si in fillers:
                    fillers[si]()
            elif prev is not None:
                # tail: PV for the final step (no next step to interleave
                # with); drain any remaining deferred work between tiles.
                for kt in range(MS):
                    for h2 in range(2):
                        emit_pv(h2, prev, av_t, s_t, kt)
                    drain_defer(kt)
            if prev is not None:
                b_p, n_p, hp_p, _ = prev
                # denominator reciprocal inline (frees the single-buffered
                # s bank before the next step's PV writes it); the rest of
                # the finalize + O projection is deferred into the NEXT
                # step's kt loop.  Finalize and O thunks are interleaved so
                # consecutive opsum allocations are spaced apart (slack for
                # the DVE evacuation before the PSUM slot is reused).
                rcp = npool.tile([128, 8], F32, tag="rcp", name="rcp")
                nc.vector.reciprocal(rcp[:], s_t[:])
                fins = make_finalize_thunks(prev, av_t, rcp)
                if hp_p == 1:
                    omms, odmas = make_o_thunks(n_p, b_p)
                    for i in range(4):
                        pe_defer.append(fins[i])
                        pe_defer.extend(omms[i])
                        if i >= 1:
                            pe_defer.extend(odmas[i - 1])
                    pe_defer.extend(odmas[3])
                else:
                    pe_defer.extend(fins)
            if step is not None:
                prev = (b, n, hp, pT)
                av_t = apsum.tile([128, 2, 4, DK], F32, tag="av", name="av")
                s_t = tpsum.tile([128, 8], F32, tag="s", name="s")
            else:
                prev = None
        # tail: flush any remaining deferred finalize / O-projection work
        while pe_defer:
            pe_defer.pop(0)()

# ====================== host wrapper ======================

def _prep_core_inputs(c, y1, y2, attn_bias, Wq, bq, Wk, bk, Wv, bv, Wo, bo):
    bp, hq = c // 4, c % 4
    bsl = slice(2 * bp, 2 * bp + 2)
    hsl = slice(DH * hq, DH * (hq + 1))
    bf16 = ml_dtypes.bfloat16
    f32 = np.float32
    bias_c = np.exp(attn_bias[0, 4 * hq:4 * hq + 4].astype(f32))
    return {
        "y1T": np.ascontiguousarray(y1[bsl].transpose(0, 2, 1)).astype(bf16),
        "y2T": np.ascontiguousarray(y2[bsl].transpose(0, 2, 1)).astype(bf16),
        "ebT": np.ascontiguousarray(bias_c.transpose(0, 2, 1)).astype(bf16),
        "idn": np.eye(128, dtype=f32).astype(bf16),
        "wq": np.ascontiguousarray(Wq[:, hsl]).astype(bf16),
        "wk": np.ascontiguousarray(Wk[:, hsl]).astype(bf16),
        "wv": np.ascontiguousarray(Wv[:, hsl]).astype(bf16),
        "wo": np.ascontiguousarray(Wo[hsl, :]).astype(bf16),
        "bqv": np.ascontiguousarray(
            (bq[hsl].astype(f32) * SCALE).reshape(2, 128).T
        ),
        "bkv": np.ascontiguousarray(bk[hsl].astype(f32).reshape(2, 128).T),
    }


def kernel(y1, y2, attn_bias, Wq, bq, Wk, bk, Wv, bv, Wo, bo):
    global _PROGRAM, LAST_RESULTS
    args = [np.asarray(x) for x in
            (y1, y2, attn_bias, Wq, bq, Wk, bk, Wv, bv, Wo, bo)]
    if _PROGRAM is None:
        _PROGRAM = build_program()
    nc, out_name = _PROGRAM

    in_maps = [_prep_core_inputs(c, *args) for c in range(N_CORES)]
    res = run_bass_kernel_spmd(nc, in_maps, list(range(N_CORES)), trace=TRACE)
    LAST_RESULTS = res

    out = np.zeros((B, Sq, DM), np.float32)
    for c in range(N_CORES):
        part = np.asarray(res.results[c][out_name]).astype(np.float32)
        bp = c // 4
        out[2 * bp] += part[0]
        out[2 * bp + 1] += part[1]
    # bias after partial-sum: bo plus the folded v-bias contribution bv @ Wo
    bias_full = (args[10].astype(np.float32)
                 + args[8].astype(np.float32) @ args[9].astype(np.float32))
    out += bias_full[None, None, :]
    return out



# revision 49
# speedup vs baseline: 1.1067x; 1.0099x over previous
"""
CrossAttention kernel for 8x Trainium2 NeuronCores (Bass/Tile).

Problem: nn_CrossAttention (B=4, Sq=Skv=1024, DM=1024, H=16, DK=64), fp32 I/O.

Sharding (Megatron-style hybrid, 8 cores = 2 batch-pairs x 4 head-quarters):
  core c -> batches {2*(c//4), 2*(c//4)+1}, heads {4*(c%4) .. 4*(c%4)+3}.
Each core computes Q/K/V projections for its 4 heads (256 of the 1024
Wq/Wk/Wv output dims), attention for those heads, and a partial O
projection (its 256 rows of Wo).  The host sums the 4 head-quarter
partials per batch and adds bo + bv@Wo (bias-after-allreduce; bv is
dropped on device since softmax weights sum to 1).

Device-side dataflow (per core), all matmuls bf16 with fp32 PSUM accumulate:
  - Host pre-transposes y1/y2 (feature-major yT, cast bf16) and ships
    ebT = exp(attn_bias)^T bf16 [HC, Skv, Sq]; no on-device casts needed.
  - qT/kT produced feature-major [dk, Sq]; bq (pre-scaled by 1/sqrt(dk)) and
    bk fused into the PSUM->SBUF evacuation via ACT activation bias.
  - v produced token-major [Skv, dk]; a ones-column is appended so the PV
    matmul emits the softmax denominator as PSUM row 64 for free (M=65).
  - logits computed transposed l^T [Skv, Sq] = k^T q; exp on ACT straight to
    bf16 in SBUF (no max-subtraction: |logits| <= ~15 is exact-exp-safe);
    the bias enters multiplicatively: p~ = exp(l) * exp(bias) via a DVE
    scalar_tensor_tensor multiply (4x mode: all-SBUF packed bf16).
  - attn^T = v^T p~^T (unnormalized) + denominator row; normalization applied
    at PSUM evacuation: r = 1/s broadcast across partitions via a DRAM
    round-trip DMA (0-step partition AP), DVE multiply during evacuation.
  - O projection token-major from assembled attn^T head-pair tiles.
"""

import os
import sys

sys.path.insert(0, "/opt/trn_rl_repo")

from contextlib import ExitStack

import numpy as np
import ml_dtypes

import concourse.bass as bass
import concourse.mybir as mybir
import concourse.tile as tile
from concourse import bacc
from concourse.bass_utils import run_bass_kernel_spmd

BF16 = mybir.dt.bfloat16
F32 = mybir.dt.float32
FP16 = mybir.dt.float16
AF = mybir.ActivationFunctionType
ALU = mybir.AluOpType

B, Sq, Skv, DM = 4, 1024, 1024, 1024
H, DK = 16, 64
HC = 4            # heads per core
DH = HC * DK      # head dims per core (256)
BC = 2            # batches per core
SCALE = DK ** -0.5
N_CORES = 8

_PROGRAM = None   # cached (nc, out_name)
TRACE = bool(int(os.environ.get("BASS_KERNEL_TRACE", "0")))
LPS_BUFS = int(os.environ.get("K_LPS_BUFS", "2"))
OPS_BUFS = int(os.environ.get("K_OPS_BUFS", "2"))
TT_POOL = int(os.environ.get("K_TT_POOL", "3"))      # exp-mults per step on gpsimd (of 16)
OEVAC_POOL = int(os.environ.get("K_OEVAC_POOL", "0"))  # 1: alternate DVE/Pool O evac
VEVAC_POOL = int(os.environ.get("K_VEVAC_POOL", "0"))  # V-proj evac on gpsimd
QKEVAC_POOL = int(os.environ.get("K_QKEVAC_POOL", "0"))  # b1 qk evac on gpsimd
FINCP_POOL = int(os.environ.get("K_FINCP_POOL", "0"))   # pair-copy on gpsimd
LAST_RESULTS = None


def build_program():
    """Build the per-core SPMD Bass program (identical on all 8 cores)."""
    nc = bacc.Bacc(
        "TRN2",
        target_bir_lowering=False,
        debug=False,
        num_devices=N_CORES,
    )

    # ---- DRAM parameters (per-core shards, host-prepared) ----
    y1T = nc.dram_tensor("y1T", [BC, DM, Skv], BF16, kind="ExternalInput")
    y2T = nc.dram_tensor("y2T", [BC, DM, Sq], BF16, kind="ExternalInput")
    ebT = nc.dram_tensor("ebT", [HC, Skv, Sq], BF16, kind="ExternalInput")  # biasT bf16
    idn = nc.dram_tensor("idn", [128, 128], BF16, kind="ExternalInput")  # identity
    wq = nc.dram_tensor("wq", [DM, DH], BF16, kind="ExternalInput")
    wk = nc.dram_tensor("wk", [DM, DH], BF16, kind="ExternalInput")
    wv = nc.dram_tensor("wv", [DM, DH], BF16, kind="ExternalInput")
    wo = nc.dram_tensor("wo", [DH, DM], BF16, kind="ExternalInput")
    bqv = nc.dram_tensor("bqv", [128, 2], F32, kind="ExternalInput")   # bq*SCALE, col-chunked
    bkv = nc.dram_tensor("bkv", [128, 2], F32, kind="ExternalInput")   # bk, col-chunked
    out = nc.dram_tensor("out", [BC, Sq, DM], BF16, kind="ExternalOutput")

    with tile.TileContext(nc) as tc, ExitStack() as ctx:
        build_kernel(ctx, tc, y1T, y2T, ebT, wq, wk, wv, wo, bqv, bkv, idn, out)

    nc.compile()
    return nc, "out"


def build_kernel(ctx, tc, y1T, y2T, ebT, wq, wk, wv, wo, bqv, bkv, idn, out):
    nc = tc.nc
    KT = DM // 128            # 8 contraction tiles for projections
    KG = 2                    # k-tiles per y DMA group
    NG = KT // KG             # 4 groups
    NQ = Sq // 512            # 2 moving-dim halves
    MS = Skv // 128           # 8 skv row tiles

    # ---------------- constant / weight loads ----------------
    consts = ctx.enter_context(tc.tile_pool(name="consts", bufs=1))

    wq_sb = consts.tile([128, KT, DH], BF16, tag="wq", name="wq")
    wk_sb = consts.tile([128, KT, DH], BF16, tag="wk", name="wk")
    wv_sb = consts.tile([128, KT, DH], BF16, tag="wv", name="wv")
    wo_sb = consts.tile([128, 2, DM], BF16, tag="wo", name="wo")   # [256,1024] -> 2 k-tiles
    bq_sb = consts.tile([128, 2], F32, tag="bq", name="bq")
    bk_sb = consts.tile([128, 2], F32, tag="bk", name="bk")
    idn_sb = consts.tile([128, 128], BF16, tag="idn", name="idn")
    ones_sb = consts.tile([128, 1], BF16, tag="ones", name="ones")
    nc.vector.memset(ones_sb[:], 1.0)

    # wq/bq issued first so the Q projection can start ASAP (wq in two chunks
    # for finer arrival granularity); the rest of the weight loads are
    # interleaved into the phase-P DMA stream (HWDGE is a serialized
    # resource, so issue order = arrival order).
    wq_r = wq.ap().rearrange("(k p) m -> p k m", p=128)
    nc.sync.dma_start(out=wq_sb[:, 0:1, :], in_=wq_r[:, 0:1, :])

    def load_v_weight():
        nc.sync.dma_start(out=wv_sb[:], in_=wv.ap().rearrange("(k p) m -> p k m", p=128))

    def load_o_weight():
        nc.sync.dma_start(out=wo_sb[:], in_=wo.ap().rearrange("(k p) m -> p k m", p=128))

    def load_idn():
        nc.sync.dma_start(out=idn_sb[:], in_=idn[:, :])

    def load_k_weights():
        nc.sync.dma_start(out=wk_sb[:], in_=wk.ap().rearrange("(k p) m -> p k m", p=128))
        nc.sync.dma_start(out=bk_sb[:], in_=bkv[:, :])

    # persistent activations (both batches)
    acts = ctx.enter_context(tc.tile_pool(name="acts", bufs=1))
    qT_sb = [[acts.tile([128, Sq], BF16, tag=f"qT{b}{p}", name=f"qT{b}{p}") for p in range(2)]
             for b in range(BC)]                       # [b][head-pair] rows=2x64 dk
    kT_sb = [[acts.tile([128, Skv], BF16, tag=f"kT{b}{p}", name=f"kT{b}{p}") for p in range(2)]
             for b in range(BC)]
    v_sb = [[acts.tile([128, HC, DK], BF16, tag=f"v{b}{m}", name=f"v{b}{m}") for m in range(MS)]
            for b in range(BC)]                        # [b][skv-tile][head][dk]
    pair_sb = [[acts.tile([128, Sq], BF16, tag=f"at{b}{p}", name=f"at{b}{p}") for p in range(2)]
               for b in range(BC)]                     # attnT head-pair tiles

    # ---------------- Phase P: batch-0 Q/K projections ----------------
    # Batch-1's Q/K (and both V projections) are deferred into batch-0's
    # attention steps, where the ACT-paced exp stream leaves the PE idle.
    y1g_all = {}
    y2g_all = {}
    # ybf1 fully double-buffered: y1 batch-1 loads must not carry WAR waits
    # (a waiting DMA at the SP queue head blocks every later DMA).
    ybf1 = ctx.enter_context(tc.tile_pool(name="ybf1", bufs=2 * NG))
    ybf = ctx.enter_context(tc.tile_pool(name="ybf", bufs=int(os.environ.get("K_Y_BUFS", "6"))))
    def load_y(dram, pool, b, tag, split_first=False, specs=None):
        # split_first: first contraction group loaded as two single-k DMAs so
        # the first projection matmul can start ~1.4us earlier at startup.
        if specs is None:
            specs = ([(0, 1), (1, 1)] + [(k, KG) for k in range(KG, KT, KG)]
                     ) if split_first else [(k, KG) for k in range(0, KT, KG)]
        gs = []
        for kbase, kcnt in specs:
            t = pool.tile([128, kcnt, Sq], BF16, tag=tag, name=tag)
            nc.sync.dma_start(
                out=t[:],
                in_=dram[b, 128 * kbase:128 * (kbase + kcnt), :]
                .rearrange("(k p) q -> p k q", p=128),
            )
            gs.append((t, kbase, kcnt))
        return gs

    def yap(gs, k):
        """[128, Sq] AP for contraction tile k of a load_y result."""
        for t, kb, kc in gs:
            if kb <= k < kb + kc:
                return t[:, k - kb, :]
        raise KeyError(k)

    with ExitStack() as pctx:
        ppsum = pctx.enter_context(tc.tile_pool(name="ppsum", bufs=4, space="PSUM"))

        # DMA issue order (serialized HWDGE/DMA pipe): wq k0, y2b0 k0/k1,
        # wq rest + bq, y2b0 rest, wk, y1b0, wv -- the first Q matmul can
        # start after ~330 KB instead of ~2.6 MB.
        y2g_all[0] = load_y(y2T, ybf, 0, "ybf", specs=[(0, 1)])
        nc.sync.dma_start(out=wq_sb[:, 1:KT, :], in_=wq_r[:, 1:KT, :])
        nc.sync.dma_start(out=bq_sb[:], in_=bqv[:, :])
        y2g_all[0] += load_y(y2T, ybf, 0, "ybf",
                             specs=[(1, 1)] + [(k, KG) for k in range(KG, KT, KG)])
        load_k_weights()
        qps = {(m, n): ppsum.tile([128, 512], F32, tag="ps", name="ps")
               for m in range(2) for n in range(NQ)}
        for k in range(KT):           # k-outer: start as soon as group lands
            for m in range(2):
                for n in range(NQ):
                    nc.tensor.matmul(
                        qps[(m, n)][:],
                        lhsT=wq_sb[:, k, 128 * m:128 * (m + 1)],
                        rhs=yap(y2g_all[0], k)[:, 512 * n:512 * (n + 1)],
                        start=(k == 0), stop=(k == KT - 1),
                    )
        for m in range(2):
            for n in range(NQ):
                nc.scalar.activation(
                    qT_sb[0][m][:, 512 * n:512 * (n + 1)], qps[(m, n)][:],
                    AF.Identity, bias=bq_sb[:, m:m + 1], scale=SCALE,
                )
        y1g_all[0] = load_y(y1T, ybf1, 0, "ybf1", split_first=True)
        load_v_weight()
        kps = {(m, n): ppsum.tile([128, 512], F32, tag="ps", name="ps")
               for m in range(2) for n in range(NQ)}
        for k in range(KT):
            for m in range(2):
                for n in range(NQ):
                    nc.tensor.matmul(
                        kps[(m, n)][:],
                        lhsT=wk_sb[:, k, 128 * m:128 * (m + 1)],
                        rhs=yap(y1g_all[0], k)[:, 512 * n:512 * (n + 1)],
                        start=(k == 0), stop=(k == KT - 1),
                    )
        for m in range(2):
            for n in range(NQ):
                nc.scalar.activation(
                    kT_sb[0][m][:, 512 * n:512 * (n + 1)], kps[(m, n)][:],
                    AF.Identity, bias=bk_sb[:, m:m + 1], scale=1.0,
                )
        # batch-1 y loads are issued inside the attention phase (steps 0-1)
        # to keep them out of the startup DMA flood

    # ---------------- Phase A: attention + interleaved O projection ----------------
    with ExitStack() as actx:
        EBP = int(os.environ.get("K_EB_BUFS", "6"))
        ebpool = actx.enter_context(tc.tile_pool(name="eb", bufs=EBP))
        lpsum = actx.enter_context(tc.tile_pool(name="lpsum", bufs=LPS_BUFS, space="PSUM"))
        apsum = actx.enter_context(tc.tile_pool(name="apsum", bufs=2, space="PSUM"))
        PPOOL_BUFS = int(os.environ.get("K_PPOOL_BUFS", "36"))
        ppool = actx.enter_context(tc.tile_pool(name="pT", bufs=PPOOL_BUFS))
        NP = int(os.environ.get("K_NORM_BUFS", "6"))
        npool = actx.enter_context(tc.tile_pool(name="norm", bufs=NP))
        opsum = actx.enter_context(tc.tile_pool(name="opsum", bufs=OPS_BUFS, space="PSUM"))
        tpsum = actx.enter_context(tc.tile_pool(name="tpsum", bufs=1, space="PSUM"))
        opool = actx.enter_context(tc.tile_pool(name="osb", bufs=int(os.environ.get("K_O_BUFS", "6"))))

        def emit_pv(h2, st, avb, s_t, kt):
            # transposed PV: av[sq-tile, 64] += pT_slice.T @ v;  denominator
            # s[sq] += pT_slice.T @ ones into the shared s bank.
            # start/stop are bank-level: start=True clears has_written for the
            # whole bank, so only the first/last matmul into each bank carry
            # them; untouched elements store (not accumulate).
            b, n_p, hp_p, pT_p = st
            h = 2 * hp_p + h2
            for q in range(4):
                nc.tensor.matmul(
                    avb[:, h2, q, :],
                    lhsT=pT_p[(b, kt, h2)][:, 128 * q:128 * (q + 1)],
                    rhs=v_sb[b][kt][:, h, :],
                    start=(kt == 0 and h2 == 0 and q == 0),
                    stop=(kt == MS - 1 and h2 == 1 and q == 3),
                    skip_group_check=True,
                )
                si_ = 4 * h2 + q
                nc.tensor.matmul(
                    s_t[:, si_:si_ + 1],
                    lhsT=pT_p[(b, kt, h2)][:, 128 * q:128 * (q + 1)],
                    rhs=ones_sb[:],
                    start=(kt == 0 and h2 == 0 and q == 0),
                    stop=(kt == MS - 1 and h2 == 1 and q == 3),
                    skip_group_check=True,
                )

        # Deferred-PE-work queue: finalize / O-projection thunks for step
        # si-2 are drained inside step si's kt loop so the PE always has
        # independent work between the ACT-paced logits tiles and the
        # scalar/vector engines never drain at step boundaries.
        pe_defer = []

        def drain_defer(kt):
            remaining = MS - kt
            ndrain = (len(pe_defer) + remaining - 1) // remaining
            for _ in range(ndrain):
                if pe_defer:
                    pe_defer.pop(0)()

        def make_finalize_thunks(st, avb, rcp, tail=False):
            # per-partition normalization (r = 1/denominator computed inline
            # at the step boundary), then PE transpose back to the attnT
            # layout the O projection needs.
            b, n_p, hp_p, _ = st
            tp = [None]

            def mk(q):
                # both h2 halves staged side by side in one [128,128] block:
                # a single PE transpose + single copy lands the full pair
                # partition range for this q column-block.
                def thunk():
                    if tp[0] is None:
                        tp[0] = tpsum.tile([128, 4, 128], BF16, tag="tp", name="tp")
                    attn_n = npool.tile([128, 128], BF16, tag="an", name="an")
                    for h2 in range(2):
                        nc.vector.tensor_scalar(
                            attn_n[:, 64 * h2:64 * (h2 + 1)], avb[:, h2, q, :],
                            rcp[:, 4 * h2 + q:4 * h2 + q + 1],
                            None, op0=ALU.mult,
                        )
                    nc.tensor.transpose(tp[0][:, q, :], attn_n[:], idn_sb[:])
                    dst = pair_sb[b][hp_p][:, 512 * n_p + 128 * q:
                                           512 * n_p + 128 * (q + 1)]
                    if tail and (q % 2):
                        # ACT is idle after the last exp: offload alternate
                        # copies so the tail drains on two engines
                        nc.scalar.copy(dst, tp[0][:, q, :])
                    else:
                        nc.vector.tensor_copy(dst, tp[0][:, q, :])
                return thunk

            # q-major: the O matmul for column-block q depends on exactly the
            # preceding thunk.
            return [mk(q) for q in range(4)]

        def make_o_thunks(n, b, tail=False):
            # O projection for sq half n.  The out-store dma_start is a
            # SEPARATE thunk scheduled after all four compute thunks: by the
            # time it issues, its o_t is (nearly) ready, so it does not park
            # at the head of the in-order SP DMA queue blocking later loads.
            o_ts = {}

            def mk(mt, no):
                def thunk():
                    if no == 0:
                        o_ts[mt] = opool.tile([128, DM], BF16, tag="osb", name="osb")
                    o_t = o_ts[mt]
                    ps = opsum.tile([128, 512], F32, tag="o", name="o")
                    for kp in range(2):
                        nc.tensor.matmul(
                            ps[:],
                            lhsT=pair_sb[b][kp][:, 128 * mt:128 * (mt + 1)],
                            rhs=wo_sb[:, kp, 512 * no:512 * (no + 1)],
                            start=(kp == 0), stop=(kp == 1),
                        )
                    i_e = (mt % 2) if int(os.environ.get("K_OEVAC_MT", "0")) else (no + 2 * mt) % 2
                    if tail:
                        eng = nc.scalar if i_e else nc.vector
                    elif OEVAC_POOL == 0:
                        eng = nc.vector
                    elif OEVAC_POOL == 1:
                        eng = nc.gpsimd if i_e else nc.vector
                    elif OEVAC_POOL == 2:
                        eng = nc.scalar if i_e else nc.vector
                    else:
                        eng = (nc.vector, nc.gpsimd, nc.scalar)[(no + 2 * mt) % 3]
                    if eng is nc.scalar:
                        eng.copy(o_t[:, 512 * no:512 * (no + 1)], ps[:])
                    else:
                        eng.tensor_copy(o_t[:, 512 * no:512 * (no + 1)], ps[:])
                return thunk

            def mk_dma(mt, no):
                def thunk():
                    nc.sync.dma_start(
                        out=out[b, 128 * mt:128 * (mt + 1),
                                512 * no:512 * (no + 1)],
                        in_=o_ts[mt][:, 512 * no:512 * (no + 1)],
                    )
                return thunk

            mts = list(range(4 * n, 4 * (n + 1)))
            return ([[mk(mt, 0), mk(mt, 1)] for mt in mts],
                    [[mk_dma(mt, 0), mk_dma(mt, 1)] for mt in mts])

        def emit_v_proj(vb):
            # deferred V projection: PE filler for the ACT-paced exp stream
            for mt in range(MS):
                vps = opsum.tile([128, DH], F32, tag="o", name="psv")
                for k in range(KT):
                    nc.tensor.matmul(
                        vps[:],
                        lhsT=yap(y1g_all[vb], k)[:, 128 * mt:128 * (mt + 1)],
                        rhs=wv_sb[:, k, :],
                        start=(k == 0), stop=(k == KT - 1),
                    )
                eng = nc.gpsimd if VEVAC_POOL else nc.vector
                eng.tensor_copy(
                    v_sb[vb][mt][:],
                    vps[:].rearrange("p (h d) -> p h d", d=DK),
                )

        def emit_qk_proj(w_sb, yg, dst, bias_sb, scale):
            # batch-1 Q/K projection, m-outer so only one PSUM group is in
            # flight (shares the opsum pool); DVE/gpsimd evacuation keeps the
            # scalar engine free for the exp stream.
            for m in range(2):
                for n2 in range(NQ):
                    ps = opsum.tile([128, 512], F32, tag="o", name="qk1")
                    for k in range(KT):
                        nc.tensor.matmul(
                            ps[:],
                            lhsT=w_sb[:, k, 128 * m:128 * (m + 1)],
                            rhs=yap(yg, k)[:, 512 * n2:512 * (n2 + 1)],
                            start=(k == 0), stop=(k == KT - 1),
                        )
                    eng = nc.gpsimd if QKEVAC_POOL else nc.vector
                    eng.tensor_scalar(
                        dst[m][:, 512 * n2:512 * (n2 + 1)], ps[:],
                        scale, bias_sb[:, m:m + 1], op0=ALU.mult, op1=ALU.add,
                    )

        # Step order (n, b, hp): consecutive b-steps share the same (n, hp)
        # exp-bias tiles, halving ebT DMA traffic.  batch-1's Q/K/V
        # projections are fillers in steps 0-2 (emitted before the first b1
        # step so its logits find the tiles written).
        steps = [(b, n, hp) for n in range(NQ) for b in range(BC) for hp in range(2)]

        def filler0():
            y2g_all[1] = load_y(y2T, ybf, 1, "ybf")
            emit_v_proj(0)
            load_idn()
            y1g_all[1] = load_y(y1T, ybf1, 1, "ybf1")
            load_o_weight()

        def filler1():
            emit_qk_proj(wq_sb, y2g_all[1], qT_sb[1], bq_sb, SCALE)
            emit_qk_proj(wk_sb, y1g_all[1], kT_sb[1], bk_sb, 1.0)

        fillers = {0: filler0,
                   1: filler1,
                   2: lambda: emit_v_proj(1)}
        eb_cache = {}          # (n, hp) -> {h2: tile}
        EB_PREFETCH = {2: [(1, 0)], 3: [(1, 1)]}

        def load_eb_pair(n_l, hp_l):
            if (n_l, hp_l) in eb_cache:
                return
            eb_l = {}
            for h2 in range(2):
                ebf = ebpool.tile([128, MS, 512], BF16, tag="eb", name="eb")
                for kh in range(2):
                    nc.sync.dma_start(
                        out=ebf[:, 4 * kh:4 * (kh + 1), :],
                        in_=ebT[2 * hp_l + h2,
                                512 * kh:512 * (kh + 1),
                                512 * n_l:512 * (n_l + 1)]
                        .rearrange("(k p) q -> p k q", p=128),
                    )
                eb_l[h2] = ebf
            eb_cache[(n_l, hp_l)] = eb_l
        prev = None            # (b, n, hp, pT) of the previous step
        for si, step in enumerate(steps + [None]):
            if step is not None:
                b, n, hp = step
                # exp(bias) tiles for this step's two heads (loaded in kt
                # halves for finer arrival; reused by the later batch).  Both
                # hp pairs of an n-group are issued together so the hp=1 tiles
                # aren't queued behind the batch-1 y loads; the n=1 pairs are
                # prefetched during steps 2-3 (see EB_PREFETCH below).
                for n_l, hp_l in [(n, 0), (n, 1)] + EB_PREFETCH.get(si, []):
                    load_eb_pair(n_l, hp_l)
                eb_t = eb_cache[(n, hp)]
                pT = {}
                for kt in range(MS):
                    for h2 in range(2):
                        lps = lpsum.tile([128, 512], F32, tag="l", name="l")
                        nc.tensor.matmul(
                            lps[:],
                            lhsT=kT_sb[b][hp][64 * h2:64 * (h2 + 1),
                                              128 * kt:128 * (kt + 1)],
                            rhs=qT_sb[b][hp][64 * h2:64 * (h2 + 1),
                                             512 * n:512 * (n + 1)],
                            start=True, stop=True,
                        )
                        pt = ppool.tile([128, 512], BF16, tag="pT", name="pT")
                        nc.scalar.activation(pt[:], lps[:], AF.Exp)
                        # bias enters multiplicatively: p~ = exp(l)*exp(b).
                        # bf16 SBUF tensor_tensor runs in the DVE 2x mode; a
                        # slice of the work goes to the idle gpsimd.
                        tt_i = 2 * kt + h2
                        eng = nc.gpsimd if (tt_i * TT_POOL) // 16 != ((tt_i + 1) * TT_POOL) // 16 else nc.vector
                        eng.tensor_tensor(
                            pt[:], pt[:], eb_t[h2][:, kt, :], op=ALU.mult,
                        )
                        pT[(b, kt, h2)] = pt
                    if prev is not None:
                        for h2 in range(2):   # previous step's PV, same kt
                            emit_pv(h2, prev, av_t, s_t, kt)
                    drain_defer(kt)
                if si in fillers:
                    fillers[si]()
            elif prev is not None:
                # tail: PV for the final step (no next step to interleave
                # with); drain any remaining deferred work between tiles.
                for kt in range(MS):
                    for h2 in range(2):
                        emit_pv(h2, prev, av_t, s_t, kt)
                    drain_defer(kt)
            if prev is not None:
                b_p, n_p, hp_p, _ = prev
                # denominator reciprocal inline (frees the single-buffered
                # s bank before the next step's PV writes it); the rest of
                # the finalize + O projection is deferred into the NEXT
                # step's kt loop.  Finalize and O thunks are interleaved so
                # consecutive opsum allocations are spaced apart (slack for
                # the DVE evacuation before the PSUM slot is reused).
                rcp = npool.tile([128, 8], F32, tag="rcp", name="rcp")
                nc.vector.reciprocal(rcp[:], s_t[:])
                fins = make_finalize_thunks(prev, av_t, rcp, tail=step is None)
                if hp_p == 1:
                    omms, odmas = make_o_thunks(n_p, b_p, tail=step is None)
                    for i in range(4):
                        pe_defer.append(fins[i])
                        pe_defer.extend(omms[i])
                        if i >= 1:
                            pe_defer.extend(odmas[i - 1])
                    pe_defer.extend(odmas[3])
                else:
                    pe_defer.extend(fins)
            if step is not None:
                prev = (b, n, hp, pT)
                av_t = apsum.tile([128, 2, 4, DK], F32, tag="av", name="av")
                s_t = tpsum.tile([128, 8], F32, tag="s", name="s")
            else:
                prev = None
        # tail: flush any remaining deferred finalize / O-projection work
        while pe_defer:
            pe_defer.pop(0)()

# ====================== host wrapper ======================

def _prep_core_inputs(c, y1, y2, attn_bias, Wq, bq, Wk, bk, Wv, bv, Wo, bo):
    bp, hq = c // 4, c % 4
    bsl = slice(2 * bp, 2 * bp + 2)
    hsl = slice(DH * hq, DH * (hq + 1))
    bf16 = ml_dtypes.bfloat16
    f32 = np.float32
    bias_c = np.exp(attn_bias[0, 4 * hq:4 * hq + 4].astype(f32))
    return {
        "y1T": np.ascontiguousarray(y1[bsl].transpose(0, 2, 1)).astype(bf16),
        "y2T": np.ascontiguousarray(y2[bsl].transpose(0, 2, 1)).astype(bf16),
        "ebT": np.ascontiguousarray(bias_c.transpose(0, 2, 1)).astype(bf16),
        "idn": np.eye(128, dtype=f32).astype(bf16),
        "wq": np.ascontiguousarray(Wq[:, hsl]).astype(bf16),
        "wk": np.ascontiguousarray(Wk[:, hsl]).astype(bf16),
        "wv": np.ascontiguousarray(Wv[:, hsl]).astype(bf16),
        "wo": np.ascontiguousarray(Wo[hsl, :]).astype(bf16),
        "bqv": np.ascontiguousarray(
            (bq[hsl].astype(f32) * SCALE).reshape(2, 128).T
        ),
        "bkv": np.ascontiguousarray(bk[hsl].astype(f32).reshape(2, 128).T),
    }


def kernel(y1, y2, attn_bias, Wq, bq, Wk, bk, Wv, bv, Wo, bo):
    global _PROGRAM, LAST_RESULTS
    args = [np.asarray(x) for x in
            (y1, y2, attn_bias, Wq, bq, Wk, bk, Wv, bv, Wo, bo)]
    if _PROGRAM is None:
        _PROGRAM = build_program()
    nc, out_name = _PROGRAM

    in_maps = [_prep_core_inputs(c, *args) for c in range(N_CORES)]
    res = run_bass_kernel_spmd(nc, in_maps, list(range(N_CORES)), trace=TRACE)
    LAST_RESULTS = res

    out = np.zeros((B, Sq, DM), np.float32)
    for c in range(N_CORES):
        part = np.asarray(res.results[c][out_name]).astype(np.float32)
        bp = c // 4
        out[2 * bp] += part[0]
        out[2 * bp + 1] += part[1]
    # bias after partial-sum: bo plus the folded v-bias contribution bv @ Wo
    bias_full = (args[10].astype(np.float32)
                 + args[8].astype(np.float32) @ args[9].astype(np.float32))
    out += bias_full[None, None, :]
    return out

